# revision 1
# baseline (speedup 1.0000x reference)
"""MinCutNet (2x GCN + dense_mincut_pool losses) as an 8-core Trainium2
Bass/Tile kernel.

Sharding: nodes row-wise across 8 cores (1280 nodes/core, padded N=10240).
GCN scatter (segment_sum) runs as sorted-COO one-hot matmuls on the PE;
per-edge feature gathers use SWDGE dma_gather from core-local HBM copies of
the full activation matrix, which are refreshed between layers with
AllGather collectives. Final scalar terms reduce with a tiny AllReduce.
"""

import os
import sys

sys.path.insert(0, "/opt/trn_rl_repo")

import numpy as np

import concourse.bass as bass
import concourse.mybir as mybir
import concourse.tile as tile
from concourse import library_config
from concourse.bass_utils import run_bass_kernel_spmd
from concourse.library_overlay import lower_extended_insts
from concourse.vector_clock import ScopedClock

# ---------------------------------------------------------------- constants
N, E = 10000, 320000
FIN, FH, K = 128, 256, 64
C = 8               # cores
P = 128             # partitions
NPAD = 10240        # 80 blocks of 128
SHARD = NPAD // C   # 1280 nodes per core
BLK = SHARD // P    # 10 blocks per core
NBLK = NPAD // P    # 80 blocks total
K1 = 0              # split-AG piece sizes; 0 = single AllGather (collectives
K2 = 0              # block the Pool queue, so splitting them stalls the gathers)
F32 = mybir.dt.float32
BF16 = mybir.dt.bfloat16
I16 = mybir.dt.int16
import ml_dtypes

NPBF16 = ml_dtypes.bfloat16

_DEBUG_OUTPUTS = bool(int(os.environ.get("KERNEL_DEBUG_OUTPUTS", "0")))
_MAX_PHASE = int(os.environ.get("KERNEL_MAX_PHASE", "9"))


# ------------------------------------------------------- tile drain patch
def _patched_drain_and_barrier(self, tick_clock, wait_clock):
    """walrus in this container rejects >1 sync-wait command on the tail
    Drain; spread the waits across SP nops (1 wait each)."""
    nc = self.nc
    drain_inst = nc.sync.drain()
    wait_clock.add_sem_waits(
        drain_inst.ins, ScopedClock({None: tick_clock.global_clock})
    )
    waits = list(drain_inst.ins.sync_info.on_wait)
    if len(waits) > 1:
        upd = list(drain_inst.ins.sync_info.on_update)
        drain_inst.ins.sync_info = mybir.SyncInfo(on_wait=waits[:1], on_update=upd)
        for i, w in enumerate(waits[1:]):
            nop = nc.sync.nop(nofuse=True, hint=f"tailwait{i}")
            nop.ins.sync_info = mybir.SyncInfo(on_wait=[w], on_update=[])
    nc.all_engine_barrier()
    assert self.sems is not None
    popped = nc._tile_sem_poison_stack.pop()
    assert popped is self._sem_poison
    nc.clear_and_free_semaphores(list(self.sems.allocated().values()))
    nc.all_engine_barrier()


tile.TileContext._drain_and_barrier = _patched_drain_and_barrier

_noop_ctr = [0]


def _split_excess_waits(nc, lim=1):
    """walrus in this container caps sync-wait commands per instruction;
    spill excess waits onto same-engine NOPs placed just before."""
    nsplit = 0
    for fn in nc.m.functions:
        for b in fn.blocks:
            newl = []
            changed = False
            for inst in b.instructions:
                si = inst.sync_info
                if si is not None and len(si.on_wait) > lim:
                    waits = list(si.on_wait)
                    head, tail = waits[: len(waits) - lim], waits[len(waits) - lim :]
                    for i in range(0, len(head), lim):
                        _noop_ctr[0] += 1
                        nop = mybir.InstNoOp(
                            name=f"waitnop-{_noop_ctr[0]}",
                            sync_info=mybir.SyncInfo(
                                on_wait=head[i : i + lim], on_update=[]
                            ),
                            bass_nofuse=True,
                            engine=inst.engine,
                        )
                        newl.append(nop)
                    inst.sync_info = mybir.SyncInfo(
                        on_wait=tail, on_update=list(si.on_update)
                    )
                    nsplit += 1
                    changed = True
                newl.append(inst)
            if changed:
                b.instructions = newl
    return nsplit


# ------------------------------------------------------- host preprocessing
def _bucket_edges(src, dst, w, ntiles):
    """Partition edges by 128-node dst block; pad each (core, block) bucket
    to ntiles*128 entries. Returns per-core [BLK, T*128] arrays."""
    T = ntiles
    a_src = np.zeros((C, BLK, T * P), np.int16)
    a_dloc = np.zeros((C, BLK, T * P), np.float32)
    a_w = np.zeros((C, BLK, T * P), np.float32)
    blk = dst // P
    order = np.argsort(blk, kind="stable")
    src, dst, w, blk = src[order], dst[order], w[order], blk[order]
    counts = np.bincount(blk, minlength=NBLK)
    starts = np.concatenate([[0], np.cumsum(counts)])
    for b in range(NBLK):
        c, lb = divmod(b, BLK)
        s, e = starts[b], starts[b + 1]
        n = e - s
        a_src[c, lb, :n] = src[s:e]
        a_dloc[c, lb, :n] = (dst[s:e] - b * P).astype(np.float32)
        a_w[c, lb, :n] = w[s:e]
    return a_src, a_dloc, a_w


def _idx_layout(a_src, T):
    """[C, BLK, T*128] int16 -> dma_gather idx tables [C, 128, BLK*T*8]."""
    out = np.zeros((C, P, BLK * T * 8), np.int16)
    for c in range(C):
        for b in range(BLK):
            arr = a_src[c, b]  # [T*128]
            tab = arr.reshape(T * 8, 16).T  # [16, T*8]; idx i -> [i%16, i//16]
            out[c, :, b * T * 8 : (b + 1) * T * 8] = np.tile(tab, (8, 1))
    return out


def _tile_layout(a, T):
    """[C, BLK, T*128] f32 -> [C, 128, BLK*T] with [p, b*T+t] = a[c,b,t*128+p]."""
    return np.ascontiguousarray(
        a.reshape(C, BLK, T, P).transpose(0, 3, 1, 2).reshape(C, P, BLK * T)
    )


def _run_table(dst, w, L, self_loop):
    """Padded per-dst weight runs [NPAD, L]."""
    tab = np.zeros((NPAD, L), np.float32)
    order = np.argsort(dst, kind="stable")
    dsts, ws = dst[order], w[order]
    counts = np.bincount(dsts, minlength=NPAD)
    starts = np.concatenate([[0], np.cumsum(counts)])[:-1]
    pos = np.arange(len(dsts)) - starts[dsts]
    tab[dsts, pos] = ws
    if self_loop:
        tab[np.arange(NPAD), counts] = 1.0
    return tab


def _shard_rows(a):
    """[NPAD, L] -> per-core [C, 128, BLK*L] ([p, b*L+j] = a[c*1280+b*128+p, j])."""
    L = a.shape[1]
    return np.ascontiguousarray(
        a.reshape(C, BLK, P, L).transpose(0, 2, 1, 3).reshape(C, P, BLK * L)
    )


def preprocess(edge_index, edge_weight):
    row = edge_index[0].astype(np.int64)
    col = edge_index[1].astype(np.int64)
    ew = edge_weight.astype(np.float32)

    # GCN message-passing tables (edges + self loops), bucketed by col (dst)
    loops = np.arange(N, dtype=np.int64)
    gsrc = np.concatenate([row, loops])
    gdst = np.concatenate([col, loops])
    gw = np.concatenate([ew, np.ones(N, np.float32)])
    gcnt = np.bincount(gdst // P, minlength=NBLK)
    TG = int(np.ceil(gcnt.max() / P))
    g_src, g_dloc, g_w = _bucket_edges(gsrc, gdst, gw, TG)

    # pool tables: adj@s -> gather col, scatter row (raw edges only)
    pcnt = np.bincount(row // P, minlength=NBLK)
    TP = int(np.ceil(max(pcnt.max(), 1) / P))
    p_src, p_dloc, p_w = _bucket_edges(col.astype(np.int64), row, ew, TP)

    # degree run tables (raw edges; self-loop weight 1 appended per node)
    LC = int(np.bincount(col, minlength=NPAD).max()) + 1  # + self-loop slot
    deg_tab = _run_table(col, ew, LC, self_loop=True)  # pad nodes get deg=1
    LR = max(int(np.bincount(row, minlength=NPAD).max()), 1)
    rowdeg_tab = _run_table(row, ew, LR, self_loop=False)

    mask = np.zeros((NPAD,), np.float32)
    mask[:N] = 1.0

    deg_full = np.ascontiguousarray(
        deg_tab.reshape(NBLK, P, LC).transpose(1, 0, 2).reshape(P, NBLK * LC)
    )

    # Split-AllGather row permutations: piece 1 = first kb blocks of every
    # core's shard (rank-major), piece 2 = the rest.
    def split_rowof(n, kb):
        c, loc = n // SHARD, n % SHARD
        cut = kb * P
        return np.where(
            loc < cut,
            c * cut + loc,
            C * cut + c * (SHARD - cut) + (loc - cut),
        )

    g_src2 = split_rowof(g_src.astype(np.int64), K1).astype(np.int16)
    p_src2 = split_rowof(p_src.astype(np.int64), K2).astype(np.int16)
    tabs = dict(
        TG=TG,
        TP=TP,
        LC=LC,
        LR=LR,
        g_idx=_idx_layout(g_src, TG),
        g_idx2=_idx_layout(g_src2, TG),
        g_dloc=_tile_layout(g_dloc, TG),
        g_w=_tile_layout(g_w, TG),
        p_idx=_idx_layout(p_src2, TP),
        p_dloc=_tile_layout(p_dloc, TP),
        p_w=_tile_layout(p_w, TP),
        deg=_shard_rows(deg_tab).astype(NPBF16),
        deg_full=deg_full.astype(NPBF16),
        rowdeg=_shard_rows(rowdeg_tab),
        mask=_shard_rows(mask[:, None]),  # [C, 128, BLK]
    )
    return tabs


# --------------------------------------------------------- device program
def build_program(TG, TP, LC, LR, for_sim=False):
    nc = bass.Bass(num_devices=C)
    dp = nc.declare_dram_parameter

    x_fl = dp("x_full", [NPAD, FIN], BF16, isOutput=False)
    w1 = dp("W1", [FIN, FH], F32, isOutput=False)
    w2 = dp("W2", [FH, FH], F32, isOutput=False)
    wp = dp("Wp", [FH, K], F32, isOutput=False)
    b1 = dp("b1", [1, FH], F32, isOutput=False)
    b2 = dp("b2", [1, FH], F32, isOutput=False)
    bp = dp("bp", [1, K], F32, isOutput=False)
    g_idx = dp("g_idx", [P, BLK * TG * 8], I16, isOutput=False)
    g_idx2 = dp("g_idx2", [P, BLK * TG * 8], I16, isOutput=False)
    g_dloc = dp("g_dloc", [P, BLK * TG], F32, isOutput=False)
    g_w = dp("g_w", [P, BLK * TG], F32, isOutput=False)
    p_idx = dp("p_idx", [P, BLK * TP * 8], I16, isOutput=False)
    p_dloc = dp("p_dloc", [P, BLK * TP], F32, isOutput=False)
    p_w = dp("p_w", [P, BLK * TP], F32, isOutput=False)
    deg_t = dp("deg", [P, BLK * LC], BF16, isOutput=False)
    degf_t = dp("deg_full", [P, NBLK * LC], BF16, isOutput=False)
    rowdeg_t = dp("rowdeg", [P, BLK * LR], F32, isOutput=False)
    mask_t = dp("mask", [P, BLK], F32, isOutput=False)
    iota_t = dp("iota", [P, P], F32, isOutput=False)
    iotab_t = dp("iotab", [P, P], BF16, isOutput=False)
    ident_t = dp("ident", [P, P], F32, isOutput=False)
    id64_t = dp("id64e", [K, K], F32, isOutput=False)  # I/sqrt(K)
    ones_t = dp("ones", [P, 1], F32, isOutput=False)
    ones_row_t = dp("ones_row", [1, P], F32, isOutput=False)

    out_t = dp("out", [1, 1], F32, isOutput=True)
    dbg = {}
    if _DEBUG_OUTPUTS:
        dbg["y1"] = dp("dbg_y1", [NPAD, FH], BF16, isOutput=True)
        dbg["s"] = dp("dbg_s", [NPAD, K], F32, isOutput=True)
        dbg["numden"] = dp("dbg_numden", [1, 2], F32, isOutput=True)
        dbg["ss"] = dp("dbg_ss", [K, K], F32, isOutput=True)

    # internal DRAM
    xs_full = nc.dram_tensor("xs_full", [NPAD, FIN], BF16)
    y1_in = nc.dram_tensor("y1_in", [SHARD, FH], BF16)
    y1_full = nc.dram_tensor("y1_full", [NPAD, FH], BF16, addr_space="Shared")
    s_in = nc.dram_tensor("s_in", [SHARD, K], F32)
    s_full = nc.dram_tensor("s_full", [NPAD, K], F32, addr_space="Shared")
    ar_in = nc.dram_tensor("ar_in", [K, K + 2], F32)
    ar_out = nc.dram_tensor("ar_out", [C * K, K + 2], F32, addr_space="Shared")

    rg = [list(range(C))]
    AG = lambda i, o: nc.gpsimd.collective_compute(
        "AllGather", mybir.AluOpType.bypass, replica_groups=rg, ins=[i], outs=[o]
    )

    nc.gpsimd.load_library(library_config.mlp)

    with tile.TileContext(nc) as tc:
        with (
            tc.tile_pool(name="const", bufs=1) as cp,
            tc.tile_pool(name="tabs", bufs=1) as tp,
            tc.tile_pool(name="msg", bufs=3) as mp,
            tc.tile_pool(name="wt", bufs=10) as wtp,
            tc.tile_pool(name="work", bufs=2) as wk,
            tc.tile_pool(name="acc", bufs=1) as accp,
            tc.tile_pool(name="ps", bufs=2, space="PSUM") as ps,
            tc.tile_pool(name="psa", bufs=1, space="PSUM") as psa,
        ):
            # ---------------- constants / tables into SBUF
            def load(pool, name, src, shape, dtype=F32, eng=None):
                t = pool.tile(shape, dtype, tag=name)
                (eng or nc.sync).dma_start(out=t[:], in_=src)
                return t

            # deg_full first, on the ACT HWDGE queue: it gates dis -> xs ->
            # everything, while the SP queue drains the big edge tables.
            degf_sb = load(
                tp, "degftab", degf_t[:].rearrange("p (b l) -> p b l", l=LC),
                [P, NBLK, LC], BF16, eng=nc.scalar,
            )
            disf_sb = cp.tile([P, NBLK], F32, tag="disf")
            nc.vector.tensor_reduce(
                disf_sb[:], degf_sb[:], axis=mybir.AxisListType.X,
                op=mybir.AluOpType.add,
            )
            nc.scalar.sqrt(disf_sb[:], disf_sb[:])
            nc.vector.reciprocal(disf_sb[:], disf_sb[:])

            iota_sb = load(cp, "iota", iota_t[:], [P, P])
            iotab_sb = load(cp, "iotab", iotab_t[:], [P, P], BF16)
            ident_sb = load(cp, "ident", ident_t[:], [P, P])
            id64_sb = load(cp, "id64", id64_t[:], [K, K])
            ones_sb = load(cp, "ones", ones_t[:], [P, 1])
            ones_row_sb = load(cp, "ones_row", ones_row_t[:], [1, P])
            w1_sb = load(cp, "w1", w1[:], [P, FH])
            w2_sb = load(cp, "w2", w2[:].rearrange("(c p) f -> p c f", p=P), [P, 2, FH])
            wp_sb = load(cp, "wp", wp[:].rearrange("(c p) f -> p c f", p=P), [P, 2, K])
            b1_sb = load(cp, "b1", b1[:], [1, FH])
            b2_sb = load(cp, "b2", b2[:], [1, FH])
            bp_sb = load(cp, "bp", bp[:], [1, K])
            mask_sb = load(cp, "mask", mask_t[:], [P, BLK])
            gdloc_sb = load(tp, "gdloc", g_dloc[:], [P, BLK * TG])
            gw_sb = load(tp, "gw", g_w[:], [P, BLK * TG])
            gidx_sb = load(tp, "gidx", g_idx[:], [P, BLK * TG * 8], I16)
            gidx2_sb = load(tp, "gidx2", g_idx2[:], [P, BLK * TG * 8], I16)
            pdloc_sb = load(tp, "pdloc", p_dloc[:], [P, BLK * TP])
            pw_sb = load(tp, "pw", p_w[:], [P, BLK * TP])
            pidx_sb = load(tp, "pidx", p_idx[:], [P, BLK * TP * 8], I16)

            # ---------------- deg -> dis
            deg_sb = load(
                tp, "degtab", deg_t[:].rearrange("p (b l) -> p b l", l=LC),
                [P, BLK, LC], BF16,
            )
            dis_sb = cp.tile([P, BLK], F32, tag="dis")
            nc.vector.tensor_reduce(
                dis_sb[:], deg_sb[:], axis=mybir.AxisListType.X, op=mybir.AluOpType.add
            )
            nc.scalar.sqrt(dis_sb[:], dis_sb[:])
            nc.vector.reciprocal(dis_sb[:], dis_sb[:])

            rowdeg_sb = load(
                tp, "rowdegtab", rowdeg_t[:].rearrange("p (b l) -> p b l", l=LR),
                [P, BLK, LR],
            )
            d_sb = cp.tile([P, BLK], F32, tag="d")
            nc.vector.tensor_reduce(
                d_sb[:], rowdeg_sb[:], axis=mybir.AxisListType.X, op=mybir.AluOpType.add
            )

            # ---------------- x_scaled: full, local (x and deg_full replicated)
            XCH = 20  # blocks per x-scale chunk
            x_dr = x_fl[:].rearrange("(b p) f -> p b f", p=P)
            xs_dr = xs_full[:].rearrange("(b p) f -> p b f", p=P)
            for ch in range(NBLK // XCH):
                x_sb = mp.tile([P, XCH, FIN], BF16, tag="xin")
                nc.scalar.dma_start(
                    out=x_sb[:], in_=x_dr[:, ch * XCH : (ch + 1) * XCH, :]
                )
                xs_sb = mp.tile([P, XCH, FIN], BF16, tag="xs")
                for j in range(XCH):
                    B = ch * XCH + j
                    if j % 2 == 0:
                        nc.vector.tensor_scalar_mul(
                            xs_sb[:, j, :], x_sb[:, j, :], disf_sb[:, B : B + 1]
                        )
                    else:
                        nc.scalar.activation(
                            xs_sb[:, j, :], x_sb[:, j, :],
                            mybir.ActivationFunctionType.Copy,
                            scale=disf_sb[:, B : B + 1],
                        )
                nc.sync.dma_start(
                    out=xs_dr[:, ch * XCH : (ch + 1) * XCH, :], in_=xs_sb[:]
                )

            # ---------------- shared per-layer machinery
            def scatter_layer(src_dram, Fsrc, idx_sb, dloc_sb, w_sb, T, b, dt, io):
                """Gather block b's edge sources and scatter-accumulate into
                PSUM [128 dst, Fsrc] via one-hot matmuls. Returns psum tile."""
                msg = mp.tile([P, T, Fsrc], dt, tag="msg")
                nc.gpsimd.dma_gather(
                    msg[:],
                    src_dram,
                    idx_sb[:, b * T * 8 : (b + 1) * T * 8],
                    T * P,
                    T * P,
                    Fsrc,
                    single_packet=False,
                )
                psum = ps.tile([P, Fsrc], F32, tag="scat")
                for t in range(T):
                    wt = wtp.tile([P, P], dt, tag="onehot")
                    nc.vector.tensor_scalar(
                        wt[:],
                        io[:],
                        dloc_sb[:, b * T + t : b * T + t + 1],
                        w_sb[:, b * T + t : b * T + t + 1],
                        op0=mybir.AluOpType.is_equal,
                        op1=mybir.AluOpType.mult,
                    )
                    nc.tensor.matmul(
                        psum[:],
                        wt[:],
                        msg[:, t, :],
                        start=(t == 0),
                        stop=(t == T - 1),
                    )
                return psum

            def dense_after_scatter(psum_scat, Fsrc, wchunks_sb, Fout, bias_sb, b):
                """out_psum [128n, Fout] = (dis*psum_scat) @ W + bias."""
                sc = wk.tile([P, Fsrc], F32, tag="sc")
                nc.vector.tensor_scalar_mul(sc[:], psum_scat[:], dis_sb[:, b : b + 1])
                nch = Fsrc // P
                h_psum = ps.tile([P, Fout], F32, tag="mm")
                for c_ in range(nch):
                    tr = ps.tile([P, P], F32, tag="tr")
                    nc.tensor.transpose(
                        tr[:], sc[:, c_ * P : (c_ + 1) * P], ident_sb[:]
                    )
                    tr_sb = wk.tile([P, P], F32, tag="tr_sb")
                    nc.vector.tensor_copy(tr_sb[:], tr[:])
                    rhs = (
                        wchunks_sb[:, c_, :] if nch > 1 else wchunks_sb[:, :Fout]
                    )
                    nc.tensor.matmul(
                        h_psum[:], tr_sb[:], rhs, start=(c_ == 0), stop=False
                    )
                nc.tensor.matmul(
                    h_psum[:], ones_row_sb[:], bias_sb[:], start=False, stop=True
                )
                return h_psum

            # ---------------- layer 1
            y1_sb = wk.tile([P, BLK, FH], BF16, tag="y1")
            nc.vector.memset(y1_sb[:], 0.0)
            y1_dr = y1_in[:].rearrange("(b p) f -> p b f", p=P)
            if _MAX_PHASE >= 2:
                for b in range(BLK):
                    psc = scatter_layer(
                        xs_full[:], FIN, gidx_sb, gdloc_sb, gw_sb, TG, b,
                        BF16, iotab_sb,
                    )
                    h1 = dense_after_scatter(psc, FIN, w1_sb, FH, b1_sb, b)
                    nc.scalar.activation(
                        y1_sb[:, b, :],
                        h1[:],
                        mybir.ActivationFunctionType.Relu,
                        scale=dis_sb[:, b : b + 1],
                    )
                    if b == K1 - 1:
                        nc.sync.dma_start(
                            out=y1_dr[:, :K1, :], in_=y1_sb[:, :K1, :]
                        )
                        if _MAX_PHASE >= 3:
                            AG(y1_in[: K1 * P, :], y1_full[: C * K1 * P, :])
                nc.sync.dma_start(out=y1_dr[:, K1:, :], in_=y1_sb[:, K1:, :])
            if _MAX_PHASE >= 3:
                AG(y1_in[K1 * P :, :], y1_full[C * K1 * P :, :])
                if _DEBUG_OUTPUTS:
                    nc.sync.dma_start(out=dbg["y1"][:], in_=y1_full[:])

            # ---------------- layer 2 + softmax
            s_sb = accp.tile([P, BLK, K], F32, tag="s")
            ssq_sb = accp.tile([P, BLK], F32, tag="ssq")
            sscratch = wk.tile([P, K], F32, tag="sscratch")
            nc.vector.memset(s_sb[:], 0.0)
            nc.vector.memset(ssq_sb[:], 0.0)
            s_dr = s_in[:].rearrange("(b p) k -> p b k", p=P)
            for b in range(BLK if _MAX_PHASE >= 4 else 0):
                psc = scatter_layer(
                    y1_full[:], FH, gidx2_sb, gdloc_sb, gw_sb, TG, b, BF16, iotab_sb
                )
                h2 = dense_after_scatter(psc, FH, w2_sb, FH, b2_sb, b)
                o2 = wk.tile([P, FH], F32, tag="o2")
                nc.scalar.activation(
                    o2[:], h2[:], mybir.ActivationFunctionType.Relu
                )
                # s = softmax(o2 @ Wp + bp) * mask
                sp = ps.tile([P, K], F32, tag="mm")
                for c_ in range(2):
                    tr = ps.tile([P, P], F32, tag="tr")
                    nc.tensor.transpose(
                        tr[:], o2[:, c_ * P : (c_ + 1) * P], ident_sb[:]
                    )
                    tr_sb = wk.tile([P, P], F32, tag="tr_sb")
                    nc.vector.tensor_copy(tr_sb[:], tr[:])
                    nc.tensor.matmul(
                        sp[:], tr_sb[:], wp_sb[:, c_, :], start=(c_ == 0), stop=False
                    )
                nc.tensor.matmul(
                    sp[:], ones_row_sb[:], bp_sb[:], start=False, stop=True
                )
                smax = wk.tile([P, 1], F32, tag="smax")
                nc.vector.tensor_reduce(
                    smax[:], sp[:], axis=mybir.AxisListType.X, op=mybir.AluOpType.max,
                    negate=True,
                )
                sexp = wk.tile([P, K], F32, tag="sexp")
                ssum = wk.tile([P, 1], F32, tag="ssum")
                nc.scalar.activation(
                    sexp[:], sp[:], mybir.ActivationFunctionType.Exp,
                    bias=smax[:], accum_out=ssum[:],
                )
                nc.vector.reciprocal(ssum[:], ssum[:])
                nc.vector.tensor_scalar(
                    s_sb[:, b, :], sexp[:], ssum[:], mask_sb[:, b : b + 1],
                    op0=mybir.AluOpType.mult, op1=mybir.AluOpType.mult,
                )
                nc.scalar.activation(
                    sscratch[:], s_sb[:, b, :], mybir.ActivationFunctionType.Square,
                    accum_out=ssq_sb[:, b : b + 1],
                )
                if b == K2 - 1:
                    nc.sync.dma_start(out=s_dr[:, :K2, :], in_=s_sb[:, :K2, :])
                    if _MAX_PHASE >= 5:
                        AG(s_in[: K2 * P, :], s_full[: C * K2 * P, :])
            if _MAX_PHASE >= 4:
                nc.sync.dma_start(out=s_dr[:, K2:, :], in_=s_sb[:, K2:, :])
            if _MAX_PHASE >= 5:
                AG(s_in[K2 * P :, :], s_full[C * K2 * P :, :])
                if _DEBUG_OUTPUTS:
                    nc.sync.dma_start(out=dbg["s"][:], in_=s_full[:])

            # ---------------- pool phase: adj@s, num/den accumulators
            num_sb = accp.tile([P, BLK], F32, tag="num")
            nscratch = wk.tile([P, K], F32, tag="nscratch")
            nc.vector.memset(num_sb[:], 0.0)
            for b in range(BLK if _MAX_PHASE >= 6 else 0):
                pp = scatter_layer(
                    s_full[:], K, pidx_sb, pdloc_sb, pw_sb, TP, b, F32, iota_sb
                )
                nc.vector.tensor_tensor(
                    out=nscratch[:], in0=s_sb[:, b, :], in1=pp[:],
                    op=mybir.AluOpType.mult,
                )
                nc.vector.tensor_reduce(
                    num_sb[:, b : b + 1], nscratch[:],
                    axis=mybir.AxisListType.X, op=mybir.AluOpType.add,
                )

            if _MAX_PHASE >= 7:
                # ---------------- packed partial reduce: [ss | num | den]
                # ss partial from the LOCAL s shard (10 matmuls, no sfull DMA);
                # one AllGather (cheaper than AllReduce) + local sum of 8 chunks.
                ss_psum = psa.tile([K, K], F32, tag="ss")
                smalls = psa.tile([P, 8], F32, tag="smalls")
                for b in range(BLK):
                    nc.tensor.matmul(
                        ss_psum[:], s_sb[:, b, :], s_sb[:, b, :],
                        start=(b == 0), stop=(b == BLK - 1),
                    )
                red = wk.tile([P, 1], F32, tag="red")
                nc.vector.tensor_reduce(
                    red[:], num_sb[:], axis=mybir.AxisListType.X, op=mybir.AluOpType.add
                )
                num_ps = smalls[0:1, 0:1]
                nc.tensor.matmul(num_ps, red[:], ones_sb[:], start=True, stop=True)
                den_sb = wk.tile([P, BLK], F32, tag="den")
                nc.vector.tensor_tensor(
                    out=den_sb[:], in0=ssq_sb[:], in1=d_sb[:], op=mybir.AluOpType.mult
                )
                red2 = wk.tile([P, 1], F32, tag="red2")
                nc.vector.tensor_reduce(
                    red2[:], den_sb[:], axis=mybir.AxisListType.X, op=mybir.AluOpType.add
                )
                den_ps = smalls[0:1, 1:2]
                nc.tensor.matmul(den_ps, red2[:], ones_sb[:], start=True, stop=True)

                arbuf = wk.tile([K, K + 2], F32, tag="arbuf")
                nc.vector.memset(arbuf[:], 0.0)
                nc.vector.tensor_copy(arbuf[:, 0:K], ss_psum[:])
                nc.vector.tensor_copy(arbuf[0:1, K : K + 1], num_ps)
                nc.vector.tensor_copy(arbuf[0:1, K + 1 : K + 2], den_ps)
                nc.sync.dma_start(out=ar_in[:], in_=arbuf[:])
                AG(ar_in[:], ar_out[:])
                gath = wk.tile([K, C, K + 2], F32, tag="gath")
                nc.sync.dma_start(
                    out=gath[:], in_=ar_out[:].rearrange("(c r) f -> r c f", r=K)
                )
                acc = wk.tile([K, K + 2], F32, tag="acc")
                nc.vector.tensor_copy(acc[:], gath[:, 0, :])
                for c_ in range(1, C):
                    nc.vector.tensor_tensor(
                        out=acc[:], in0=acc[:], in1=gath[:, c_, :],
                        op=mybir.AluOpType.add,
                    )
                ss_sb = acc[:, 0:K]
                ndg_sb = acc[0:1, K : K + 2]
                if _DEBUG_OUTPUTS:
                    nc.sync.dma_start(out=dbg["ss"][:], in_=ss_sb)
                    nc.sync.dma_start(out=dbg["numden"][:], in_=ndg_sb)

                # ---------------- ortho loss + final scalar
                sq64 = wk.tile([K, K], F32, tag="sq64")
                col64 = wk.tile([K, 1], F32, tag="col64")
                nc.scalar.activation(
                    sq64[:], ss_sb, mybir.ActivationFunctionType.Square,
                    accum_out=col64[:],
                )
                fro_ps = smalls[0:1, 2:3]
                nc.tensor.matmul(fro_ps, col64[:], ones_sb[:K, :], start=True, stop=True)
                fro = wk.tile([1, 1], F32, tag="fro_sb")
                nc.scalar.sqrt(fro[:], fro_ps)
                nc.vector.reciprocal(fro[:], fro[:])
                # broadcast 1/fro to K partitions via rank-1 matmul
                fro_bc = smalls[0:K, 3:4]
                nc.tensor.matmul(
                    fro_bc, ones_row_sb[:, :K], fro[:], start=True, stop=True
                )
                fro64 = wk.tile([K, 1], F32, tag="fro64")
                nc.vector.tensor_copy(fro64[:], fro_bc)
                # t = ss/fro - I/sqrt(K)
                tmat = wk.tile([K, K], F32, tag="tmat")
                nc.vector.tensor_scalar_mul(tmat[:], ss_sb, fro64[:])
                nc.vector.tensor_tensor(
                    out=tmat[:], in0=tmat[:], in1=id64_sb[:],
                    op=mybir.AluOpType.subtract,
                )
                nc.scalar.activation(
                    sq64[:], tmat[:], mybir.ActivationFunctionType.Square,
                    accum_out=col64[:],
                )
                orth_ps = smalls[0:1, 4:5]
                nc.tensor.matmul(orth_ps, col64[:], ones_sb[:K, :], start=True, stop=True)
                orth = wk.tile([1, 1], F32, tag="orth_sb")
                nc.scalar.sqrt(orth[:], orth_ps)

                rden = wk.tile([1, 1], F32, tag="rden")
                nc.vector.reciprocal(rden[:], acc[0:1, K + 1 : K + 2])
                mloss = wk.tile([1, 1], F32, tag="mloss")
                nc.vector.tensor_tensor(
                    out=mloss[:], in0=acc[0:1, K : K + 1], in1=rden[:],
                    op=mybir.AluOpType.mult,
                )
                res = wk.tile([1, 1], F32, tag="res")
                nc.vector.tensor_tensor(
                    out=res[:], in0=orth[:], in1=mloss[:], op=mybir.AluOpType.subtract
                )
                nc.sync.dma_start(out=out_t[:], in_=res[:])
            else:
                nc.sync.dma_start(out=out_t[:], in_=dis_sb[0:1, 0:1])

    if not for_sim:
        _split_excess_waits(nc)
    lower_extended_insts(nc)
    return nc


_PROG_CACHE = {}


def _get_program(key):
    if key not in _PROG_CACHE:
        _PROG_CACHE[key] = build_program(*key)
    return _PROG_CACHE[key]


def make_in_maps(inputs, tabs):
    x = np.asarray(inputs["x"], np.float32)
    W1, W2, Wp = inputs["W1"], inputs["W2"], inputs["Wp"]
    b1, b2, bp = inputs["b1"], inputs["b2"], inputs["bp"]
    xpad = np.zeros((NPAD, FIN), np.float32)
    xpad[:N] = x
    iota = np.tile(np.arange(P, dtype=np.float32), (P, 1))
    ident = np.eye(P, dtype=np.float32)
    id64e = (np.eye(K, dtype=np.float32) / np.sqrt(np.float32(K))).astype(np.float32)
    ones = np.ones((P, 1), np.float32)

    common = dict(
        W1=np.asarray(W1, np.float32),
        W2=np.asarray(W2, np.float32),
        Wp=np.asarray(Wp, np.float32),
        b1=np.asarray(b1, np.float32).reshape(1, FH),
        b2=np.asarray(b2, np.float32).reshape(1, FH),
        bp=np.asarray(bp, np.float32).reshape(1, K),
        iota=iota,
        iotab=iota.astype(NPBF16),
        ident=ident,
        id64e=id64e,
        ones=ones,
        ones_row=np.ones((1, P), np.float32),
    )
    in_maps = []
    for c in range(C):
        in_maps.append(
            dict(
                common,
                x_full=xpad.astype(NPBF16),
                deg_full=tabs["deg_full"],
                g_idx=tabs["g_idx"][c],
                g_idx2=tabs["g_idx2"][c],
                g_dloc=tabs["g_dloc"][c],
                g_w=tabs["g_w"][c],
                p_idx=tabs["p_idx"][c],
                p_dloc=tabs["p_dloc"][c],
                p_w=tabs["p_w"][c],
                deg=tabs["deg"][c],
                rowdeg=tabs["rowdeg"][c],
                mask=tabs["mask"][c],
            )
        )
    return in_maps


def kernel(x, edge_index, edge_weight, W1, b1, W2, b2, Wp, bp):
    edge_index = np.asarray(edge_index)
    edge_weight = np.asarray(edge_weight, np.float32)
    tabs = preprocess(edge_index, edge_weight)
    nc = _get_program((tabs["TG"], tabs["TP"], tabs["LC"], tabs["LR"]))
    in_maps = make_in_maps(
        dict(x=x, W1=W1, b1=b1, W2=W2, b2=b2, Wp=Wp, bp=bp), tabs
    )
    trace = bool(int(os.environ.get("KERNEL_TRACE", "0")))
    kwargs = {}
    if trace:
        kwargs = dict(trace=True, tmpdir=os.environ.get("KERNEL_TRACE_DIR"))
    res = run_bass_kernel_spmd(nc, in_maps, core_ids=list(range(C)), **kwargs)
    if trace:
        kernel.exec_time_ns = res.exec_time_ns
        kernel.mean_exec_time_ns = res.mean_exec_time_ns
        kernel.bass_results = res
    out = res.results[0]["out"].reshape(())
    if _DEBUG_OUTPUTS:
        kernel.debug = {k: res.results[0][f"dbg_{k}"] for k in ("y1", "s", "numden", "ss")}
    return np.float32(out)


if __name__ == "__main__":
    import reference

    inputs = reference.setup_inputs()
    inputs = {k: np.asarray(v) for k, v in inputs.items()}
    got = kernel(**inputs)
    print("kernel out:", got)



# revision 60
# speedup vs baseline: 2.4711x; 2.4711x over previous
"""MinCutNet (2x GCN + dense_mincut_pool losses) as an 8-core Trainium2
Bass/Tile kernel.

Sharding / dataflow (v3):
  - L1 (aggregate x): dst-sharded. x is replicated (fp8, packed 4-per-f32
    so the SWDGE gather moves 64 elements/row), each core gathers edge
    sources for its own 1280 destination rows and scatter-accumulates via
    host-precomputed fp8 one-hot tiles with DoubleRow matmuls (256 edge
    slots per PE op). No collective.
  - L2 (aggregate y1): src-sharded. y1 is stored fp8 (256 feats = 64
    packed f32 gather elements). Each core computes PARTIAL aggregation
    sums for all 10240 destinations from its locally-owned sources, then
    one bf16 ReduceScatter sums partials and hands each core its rows.
    This replaces a 5.2MB y1 AllGather (~146us in the collective cost
    model) with a 655KB-out ReduceScatter (~31us).
  - dense + softmax: fully local per shard (W2/Wp in bf16, transposed
    dataflow so no activation transposes are needed before the matmuls).
  - pool (adj @ s): col-sharded partial sums like L2; the ss / den /
    tr(ss) scalar partials (hi/lo-split bf16 for accuracy) ride in 64
    extra rows of the same ReduceScatter payload, so the only remaining
    exchange afterwards is an 8-scalar AllGather for the mincut numerator.
  - ortho loss uses the closed form ||ss/|ss| - I/sqrt(K)||_F =
    sqrt(2 - 2 tr(ss) / (|ss|_F sqrt(K))).

One-hot scatter weight tiles (normalization folded in) are precomputed on
the host from edge_index/edge_weight only and streamed from HBM on the
otherwise-idle SP queue; gathers run on the Pool/SWDGE queue; evictions
and activations are spread across ACT and DVE so every phase is paced by
its gather-byte floor rather than a single engine.
"""

import os
import sys

sys.path.insert(0, "/opt/trn_rl_repo")

import numpy as np

import concourse.bass as bass
import concourse.mybir as mybir
import concourse.tile as tile
from concourse import library_config
from concourse.bass_utils import run_bass_kernel_spmd
from concourse.library_overlay import lower_extended_insts
from concourse.vector_clock import ScopedClock

import ml_dtypes

# ---------------------------------------------------------------- constants
N, E = 10000, 320000
FIN, FH, K = 128, 256, 64
C = 8               # cores
P = 128             # partitions
NPAD = 10240        # 80 blocks of 128
SHARD = NPAD // C   # 1280 nodes per core
BLK = SHARD // P    # 10 blocks per core
NBLK = NPAD // P    # 80 blocks total
F32 = mybir.dt.float32
BF16 = mybir.dt.bfloat16
FP8 = mybir.dt.float8e4
I16 = mybir.dt.int16
NPBF16 = ml_dtypes.bfloat16

_DEBUG_OUTPUTS = bool(int(os.environ.get("KERNEL_DEBUG_OUTPUTS", "0")))
_MAX_PHASE = int(os.environ.get("KERNEL_MAX_PHASE", "9"))


# ------------------------------------------------------- tile drain patch
def _patched_drain_and_barrier(self, tick_clock, wait_clock):
    """walrus in this container rejects >1 sync-wait command on the tail
    Drain; spread the waits across SP nops (1 wait each)."""
    nc = self.nc
    drain_inst = nc.sync.drain()
    wait_clock.add_sem_waits(
        drain_inst.ins, ScopedClock({None: tick_clock.global_clock})
    )
    waits = list(drain_inst.ins.sync_info.on_wait)
    if len(waits) > 1:
        upd = list(drain_inst.ins.sync_info.on_update)
        drain_inst.ins.sync_info = mybir.SyncInfo(on_wait=waits[:1], on_update=upd)
        for i, w in enumerate(waits[1:]):
            nop = nc.sync.nop(nofuse=True, hint=f"tailwait{i}")
            nop.ins.sync_info = mybir.SyncInfo(on_wait=[w], on_update=[])
    nc.all_engine_barrier()
    assert self.sems is not None
    popped = nc._tile_sem_poison_stack.pop()
    assert popped is self._sem_poison
    nc.clear_and_free_semaphores(list(self.sems.allocated().values()))
    nc.all_engine_barrier()


tile.TileContext._drain_and_barrier = _patched_drain_and_barrier

_noop_ctr = [0]


def _split_excess_waits(nc, lim=1):
    """walrus in this container caps sync-wait commands per instruction;
    spill excess waits onto same-engine NOPs placed just before."""
    nsplit = 0
    for fn in nc.m.functions:
        for b in fn.blocks:
            newl = []
            changed = False
            for inst in b.instructions:
                si = inst.sync_info
                if si is not None and len(si.on_wait) > lim:
                    waits = list(si.on_wait)
                    head, tail = waits[: len(waits) - lim], waits[len(waits) - lim :]
                    for i in range(0, len(head), lim):
                        _noop_ctr[0] += 1
                        nop = mybir.InstNoOp(
                            name=f"waitnop-{_noop_ctr[0]}",
                            sync_info=mybir.SyncInfo(
                                on_wait=head[i : i + lim], on_update=[]
                            ),
                            bass_nofuse=True,
                            engine=inst.engine,
                        )
                        newl.append(nop)
                    inst.sync_info = mybir.SyncInfo(
                        on_wait=tail, on_update=list(si.on_update)
                    )
                    nsplit += 1
                    changed = True
                newl.append(inst)
            if changed:
                b.instructions = newl
    return nsplit


# ------------------------------------------------------- host preprocessing
def _pack_idx(arr):
    """[NT*128] int -> dma_gather idx table [128, NT*8] int16.
    idx i lives at [i % 16, i // 16], replicated over 8 partition groups."""
    nt8 = arr.shape[0] // 16
    tab = arr.astype(np.int16).reshape(nt8, 16).T          # [16, NT*8]
    return np.ascontiguousarray(np.tile(tab, (8, 1)))      # [128, NT*8]


def _pack_val(arr):
    """[NT*128] f32 -> [128, NT] tile-major: [p, t] = arr[t*128 + p]."""
    nt = arr.shape[0] // P
    return np.ascontiguousarray(arr.reshape(nt, P).T)


def _bucketize(src, dst, w, owner, nbkt, bkt_of_dst, even=False):
    """Partition edges into per-(core, bucket) slot arrays.

    owner[e]     : core that processes edge e
    bkt_of_dst[e]: bucket (0..nbkt-1) within the core's loop
    Returns (T[nbkt] shared tile counts, per-core [C, NT*128] idx/dloc/w).
    """
    cnt = np.zeros((C, nbkt), np.int64)
    np.add.at(cnt, (owner, bkt_of_dst), 1)
    T = np.maximum(1, np.ceil(cnt.max(axis=0) / P).astype(np.int64))
    if even:
        T = ((T + 1) // 2) * 2
    off = np.concatenate([[0], np.cumsum(T)])
    NT = int(off[-1])
    a_idx = np.zeros((C, NT * P), np.int64)
    a_dloc = np.zeros((C, NT * P), np.float32)
    a_w = np.zeros((C, NT * P), np.float32)
    order = np.lexsort((bkt_of_dst, owner))
    src_s, dst_s, w_s = src[order], dst[order], w[order]
    own_s, bkt_s = owner[order], bkt_of_dst[order]
    # slot position within bucket
    lin = own_s * nbkt + bkt_s
    starts = np.searchsorted(lin, np.arange(C * nbkt))
    pos = np.arange(len(lin)) - starts[lin]
    slot = off[bkt_s] * P + pos
    a_idx[own_s, slot] = src_s
    a_dloc[own_s, slot] = (dst_s % P).astype(np.float32)
    a_w[own_s, slot] = w_s
    return T, a_idx, a_dloc, a_w


def _wt_table(dloc, w, npdt):
    """[C, NT*128] dloc/w -> dense one-hot scatter tiles [C, P, NT*P]:
    tab[c][p, t*128 + d] = (dloc[c, t*128+p] == d) * w[c, t*128+p]."""
    NT = dloc.shape[1] // P
    out = np.zeros((C, P, NT * P), npdt)
    rows = np.arange(NT * P)
    for c in range(C):
        wt_c = np.zeros((NT * P, P), np.float32)
        wt_c[rows, dloc[c].astype(np.int64)] = w[c]
        out[c] = np.ascontiguousarray(
            wt_c.reshape(NT, P, P).transpose(1, 0, 2).reshape(P, NT * P)
        ).astype(npdt)
    return out


def preprocess(edge_index, edge_weight):
    row = edge_index[0].astype(np.int64)
    col = edge_index[1].astype(np.int64)
    ew = edge_weight.astype(np.float32)

    # gcn_norm on host (edge data only): deg over col with self loops
    deg = np.zeros(NPAD, np.float32)
    np.add.at(deg, col, ew)
    deg[:N] += 1.0
    deg[deg == 0] = 1.0
    dis = (1.0 / np.sqrt(deg)).astype(np.float32)

    loops = np.arange(N, dtype=np.int64)
    gsrc = np.concatenate([row, loops])
    gdst = np.concatenate([col, loops])
    gnorm = np.concatenate([dis[row] * ew * dis[col], dis[:N] ** 2]).astype(np.float32)

    # L1: dst-sharded; bucket = local dst block (0..BLK-1)
    T1, l1_idx, l1_dloc, l1_w = _bucketize(
        gsrc, gdst, gnorm, owner=gdst // SHARD, nbkt=BLK,
        bkt_of_dst=(gdst % SHARD) // P, even=True,
    )
    # L2: src-sharded; bucket = global dst block (0..NBLK-1); idx local
    T2, l2_idx, l2_dloc, l2_w = _bucketize(
        gsrc % SHARD, gdst, gnorm, owner=gsrc // SHARD, nbkt=NBLK,
        bkt_of_dst=gdst // P, even=True,
    )
    l2_wt = _wt_table(l2_dloc, l2_w, ml_dtypes.float8_e4m3)

    # pool: col-sharded; gather s[col], scatter to row buckets, w = ew
    T3, p_idx, p_dloc, p_w = _bucketize(
        col % SHARD, row, ew, owner=col // SHARD, nbkt=NBLK,
        bkt_of_dst=row // P, even=True,
    )
    l1_wt = _wt_table(l1_dloc, l1_w, ml_dtypes.float8_e4m3)
    p_wt = _wt_table(p_dloc, p_w, ml_dtypes.float8_e4m3)

    # d = row degree sums (adj.sum(-1)); per-core [128, BLK]
    d = np.zeros(NPAD, np.float32)
    np.add.at(d, row, ew)
    d_sh = d.reshape(C, BLK, P).transpose(0, 2, 1)          # [C, 128, BLK]

    mask = np.zeros(NPAD, np.float32)
    mask[:N] = 1.0
    mask_sh = mask.reshape(C, BLK, P).transpose(0, 2, 1)

    tabs = dict(
        T1=tuple(int(t) for t in T1),
        T2=tuple(int(t) for t in T2),
        T3=tuple(int(t) for t in T3),
        l1_idx=np.stack([_pack_idx(a) for a in l1_idx]),
        l1_wt=l1_wt,
        l2_idx=np.stack([_pack_idx(a) for a in l2_idx]),
        l2_wt=l2_wt,
        p_idx=np.stack([_pack_idx(a) for a in p_idx]),
        p_wt=p_wt,
        d=np.ascontiguousarray(d_sh),
        mask=np.ascontiguousarray(mask_sh),
    )
    return tabs


# --------------------------------------------------------- device program
def build_program(T1, T2, T3, for_sim=False):
    NT1, NT2, NT3 = sum(T1), sum(T2), sum(T3)
    # gather-group size: GRP buckets share one dma_gather call
    GRP = 5
    NGRP = NBLK // GRP
    GRP1 = 1
    NGRP1 = BLK // GRP1
    off1 = np.concatenate([[0], np.cumsum(T1)])
    off2 = np.concatenate([[0], np.cumsum(T2)])
    off3 = np.concatenate([[0], np.cumsum(T3)])
    SOFF1 = [int(off1[g * GRP1]) for g in range(NGRP1 + 1)]
    SOFF2 = [int(off2[g * GRP]) for g in range(NGRP + 1)]
    SOFF3 = [int(off3[g * GRP]) for g in range(NGRP + 1)]

    nc = bass.Bass(num_devices=C)
    dp = nc.declare_dram_parameter

    x_fl = dp("x_full", [NPAD, FIN // 2], F32, isOutput=False)  # 128 bf16 packed
    w1 = dp("W1", [FIN, FH], BF16, isOutput=False)           # [128, 256]
    w2 = dp("W2", [P, 2, 2, P], BF16, isOutput=False)        # [fin_p, fc, oc, fout_p]
    wp = dp("Wp", [P, 2, K], BF16, isOutput=False)           # [fout_p, oc, k]
    b1 = dp("b1", [1, FH], F32, isOutput=False)
    b2r = dp("b2r", [1, 2, P], F32, isOutput=False)          # [1, oc, fout]
    bp = dp("bp", [1, K], F32, isOutput=False)
    l1_idx = dp("l1_idx", [P, NT1 * 8], I16, isOutput=False)
    l1_wt = dp("l1_wt", [P, NT1 * P], FP8, isOutput=False)
    l2_idx = dp("l2_idx", [P, NT2 * 8], I16, isOutput=False)
    l2_wt = dp("l2_wt", [P, NT2 * P], FP8, isOutput=False)
    p_idx = dp("p_idx", [P, NT3 * 8], I16, isOutput=False)
    p_wt = dp("p_wt", [P, NT3 * P], FP8, isOutput=False)
    d_t = dp("d", [P, BLK], F32, isOutput=False)
    mask_t = dp("mask", [P, BLK], F32, isOutput=False)
    identb_t = dp("identb", [P, P], BF16, isOutput=False)
    ones_t = dp("ones", [P, 1], F32, isOutput=False)
    ones_row_t = dp("ones_row", [1, P], F32, isOutput=False)

    out_t = dp("out", [1, 1], F32, isOutput=True)
    dbg = {}
    if _DEBUG_OUTPUTS:
        dbg["y1"] = dp("dbg_y1", [SHARD, FH], FP8, isOutput=True)
        dbg["agg2"] = dp("dbg_agg2", [SHARD, FH], BF16, isOutput=True)
        dbg["s"] = dp("dbg_s", [SHARD, K], F32, isOutput=True)
        dbg["adjs"] = dp("dbg_adjs", [SHARD, K], BF16, isOutput=True)
        dbg["numden"] = dp("dbg_numden", [1, 2], F32, isOutput=True)
        dbg["ss"] = dp("dbg_ss", [K, K], F32, isOutput=True)

    # internal DRAM
    y1_in = nc.dram_tensor("y1_in", [SHARD, FH // 4], F32)  # 256 fp8 packed
    s_in = nc.dram_tensor("s_in", [SHARD, P // 2], F32)  # 256 fp8 packed
    rs1_in = nc.dram_tensor("rs1_in", [NPAD, FH], BF16)
    rs1_out = nc.dram_tensor("rs1_out", [SHARD, FH], BF16)
    CH = SHARD + K  # rows per chunk: adjs shard + packed ss/den/trss
    rs2_in = nc.dram_tensor("rs2_in", [C * CH, K + 8], BF16)
    rs2_out = nc.dram_tensor("rs2_out", [CH, K + 8], BF16)
    ar_in = nc.dram_tensor("ar_in", [K, K + 2], F32)
    ar_out = nc.dram_tensor("ar_out", [C * K, K + 2], F32, addr_space="Shared")
    ar2_in = nc.dram_tensor("ar2_in", [1, 1], F32)
    ar2_out = nc.dram_tensor("ar2_out", [C, 1], F32, addr_space="Shared")

    rg = [list(range(C))]

    def RS(i, o):
        return nc.gpsimd.collective_compute(
            "ReduceScatter", mybir.AluOpType.add, replica_groups=rg, ins=[i], outs=[o]
        )

    def AG(i, o):
        return nc.gpsimd.collective_compute(
            "AllGather", mybir.AluOpType.bypass, replica_groups=rg, ins=[i], outs=[o]
        )

    nc.gpsimd.load_library(library_config.mlp)

    with tile.TileContext(nc) as tc:
        with (
            tc.tile_pool(name="const", bufs=1) as cp,
            tc.tile_pool(name="tabs", bufs=1) as tp,
            tc.tile_pool(name="msg", bufs=3) as mp,
            tc.tile_pool(name="msg2", bufs=4) as mp2,
            tc.tile_pool(name="wt", bufs=3) as wtp,
            tc.tile_pool(name="work", bufs=3) as wk,
            tc.tile_pool(name="ev", bufs=4) as evp,
            tc.tile_pool(name="acc", bufs=1) as accp,
            tc.tile_pool(name="ps", bufs=3, space="PSUM") as ps,
            tc.tile_pool(name="psm", bufs=2, space="PSUM") as psm,
            tc.tile_pool(name="pst", bufs=2, space="PSUM") as pst,
            tc.tile_pool(name="psa", bufs=1, space="PSUM") as psa,
        ):
            # ---------------- constants / tables into SBUF
            def load(pool, name, src, shape, dtype=F32, eng=None):
                t = pool.tile(shape, dtype, tag=name)
                (eng or nc.scalar).dma_start(out=t[:], in_=src)
                return t

            identb_sb = load(cp, "identb", identb_t[:], [P, P], BF16)
            ones_sb = load(cp, "ones", ones_t[:], [P, 1])
            ones_row_sb = load(cp, "ones_row", ones_row_t[:], [1, P])
            w1_sb = load(cp, "w1", w1[:], [P, FH], BF16)
            w2_sb = load(cp, "w2", w2[:], [P, 2, 2, P], BF16)
            wp_sb = load(cp, "wp", wp[:], [P, 2, K], BF16)
            b1_sb = load(cp, "b1", b1[:], [1, FH])
            b2r_sb = load(cp, "b2r", b2r[:], [1, 2, P])
            bp_sb = load(cp, "bp", bp[:], [1, K])
            d_sb = load(cp, "d", d_t[:], [P, BLK])
            mask_sb = load(cp, "mask", mask_t[:], [P, BLK])

            l1idx_sb = load(tp, "l1idx", l1_idx[:], [P, NT1 * 8], I16, eng=nc.scalar)
            l2idx_sb = load(tp, "l2idx", l2_idx[:], [P, NT2 * 8], I16)
            pidx_sb = load(tp, "pidx", p_idx[:], [P, NT3 * 8], I16)

            l1wt_dr = l1_wt[:].rearrange("p (t d) -> p t d", d=P)
            pwt_dr = p_wt[:].rearrange("p (t d) -> p t d", d=P)

            # ---------------- L1: dst-sharded aggregate of x, then W1+relu
            y1_sb = accp.tile([P, BLK, FH], FP8, tag="y1")
            y1_dr = y1_in[:].rearrange("(b p) f -> p b f", p=P)
            for grp in range(NGRP1 if _MAX_PHASE >= 1 else 0):
                TS = SOFF1[grp + 1] - SOFF1[grp]
                msg = mp.tile([P, TS, FIN // 2], F32, tag="msg1")
                nc.gpsimd.dma_gather(
                    msg[:], x_fl[:],
                    l1idx_sb[:, SOFF1[grp] * 8 : SOFF1[grp + 1] * 8],
                    TS * P, TS * P, FIN // 2, single_packet=False,
                )
                wts = wtp.tile([P, TS, P], FP8, tag="wts1")
                nc.sync.dma_start(
                    out=wts[:], in_=l1wt_dr[:, SOFF1[grp] : SOFF1[grp + 1], :]
                )
                toff = 0
                for lb in range(GRP1):
                    b = grp * GRP1 + lb
                    T = T1[b]
                    psum = ps.tile([P, FIN], F32, tag="scat")
                    for t2 in range(T // 2):
                        nc.tensor.matmul(
                            psum[:],
                            wts[:, toff + 2 * t2 : toff + 2 * t2 + 2, :],
                            msg[:, toff + 2 * t2 : toff + 2 * t2 + 2, :].bitcast(FP8)[:, :, 0:FIN],
                            start=(t2 == 0), stop=(t2 == T // 2 - 1),
                            perf_mode=mybir.MatmulPerfMode.DoubleRow,
                        )
                    toff += T
                    # dense: y1 = relu(agg @ W1 + b1)
                    aggb = wk.tile([P, FIN], BF16, tag="aggb1")
                    nc.scalar.copy(aggb[:], psum[:])
                    aggT_ps = pst.tile([P, P], BF16, tag="tr")
                    nc.tensor.transpose(aggT_ps[:], aggb[:], identb_sb[:])
                    aggT = wk.tile([P, P], BF16, tag="aggT1s")
                    nc.vector.tensor_copy(aggT[:], aggT_ps[:])
                    h_ps = psm.tile([P, FH], F32, tag="mm")
                    nc.tensor.matmul(h_ps[:], aggT[:], w1_sb[:], start=True, stop=False)
                    nc.tensor.matmul(
                        h_ps[:], ones_row_sb[:], b1_sb[:], start=False, stop=True
                    )
                    nc.scalar.activation(
                        y1_sb[:, b, :], h_ps[:], mybir.ActivationFunctionType.Relu
                    )
                    nc.scalar.dma_start(out=y1_dr[:, b, :], in_=y1_sb[:, b, :].bitcast(F32))
            if _DEBUG_OUTPUTS and _MAX_PHASE >= 1:
                nc.sync.dma_start(out=dbg["y1"][:], in_=y1_in[:])

            # ---------------- L2: src-sharded partial aggregation + RS
            # one-hot wt tiles are host-precomputed fp8 and streamed from HBM
            rs1_dr = rs1_in[:].rearrange("(g p) f -> p g f", p=P)
            l2wt_dr = l2_wt[:].rearrange("p (t d) -> p t d", d=P)
            for grp in range(NGRP if _MAX_PHASE >= 2 else 0):
                TS = SOFF2[grp + 1] - SOFF2[grp]
                msg = mp2.tile([P, TS, FH // 4], F32, tag="msg2")
                nc.gpsimd.dma_gather(
                    msg[:], y1_in[:],
                    l2idx_sb[:, SOFF2[grp] * 8 : SOFF2[grp + 1] * 8],
                    TS * P, TS * P, FH // 4, single_packet=False,
                )
                wts = wtp.tile([P, TS, P], FP8, tag="wts2")
                nc.sync.dma_start(
                    out=wts[:], in_=l2wt_dr[:, SOFF2[grp] : SOFF2[grp + 1], :]
                )
                ev = evp.tile([P, GRP, FH], BF16, tag="evT")
                toff = 0
                for lb in range(GRP):
                    g = grp * GRP + lb
                    T = T2[g]
                    psum = ps.tile([P, FH], F32, tag="scat")
                    for t2 in range(T // 2):
                        # fp8 DoubleRow: 256 edge slots per matmul
                        nc.tensor.matmul(
                            psum[:],
                            wts[:, toff + 2 * t2 : toff + 2 * t2 + 2, :],
                            msg[:, toff + 2 * t2 : toff + 2 * t2 + 2, :].bitcast(FP8),
                            start=(t2 == 0), stop=(t2 == T // 2 - 1),
                            perf_mode=mybir.MatmulPerfMode.DoubleRow,
                        )
                    toff += T
                    if lb == 3:
                        nc.scalar.copy(ev[:, lb, :], psum[:])
                    else:
                        nc.vector.tensor_copy(ev[:, lb, :], psum[:])
                nc.scalar.dma_start(
                    out=rs1_dr[:, grp * GRP : (grp + 1) * GRP, :], in_=ev[:]
                )
            if _MAX_PHASE >= 3:
                RS(rs1_in[:], rs1_out[:])
                if _DEBUG_OUTPUTS:
                    nc.sync.dma_start(out=dbg["agg2"][:], in_=rs1_out[:])

            # ---------------- local dense: y2 = relu(agg@W2+b2); s = softmax
            s_sb = accp.tile([P, BLK, K], F32, tag="s")
            spad_sb = accp.tile([P, BLK, FH], FP8, tag="spad")
            ssq_sb = accp.tile([P, BLK], F32, tag="ssq")
            sscratch = wk.tile([P, K], F32, tag="sscratch")
            s_dr = s_in[:].rearrange("(b p) k -> p b k", p=P)
            if _MAX_PHASE >= 4:
                nc.vector.memset(spad_sb[:], 0.0)
                aggsb = accp.tile([P, BLK, FH], BF16, tag="aggsb")
                rs1o_dr = rs1_out[:].rearrange("(b p) f -> p b f", p=P)
                nc.sync.dma_start(out=aggsb[:, 0:2, :], in_=rs1o_dr[:, 0:2, :])
                nc.sync.dma_start(out=aggsb[:, 2:BLK, :], in_=rs1o_dr[:, 2:BLK, :])
                for b in range(BLK):
                    aggT2 = wk.tile([P, 2, P], BF16, tag="aggT2")
                    for fc in range(2):
                        tps = pst.tile([P, P], BF16, tag="tr")
                        nc.tensor.transpose(
                            tps[:], aggsb[:, b, fc * P : (fc + 1) * P], identb_sb[:]
                        )
                        if fc == 0:
                            nc.vector.tensor_copy(aggT2[:, fc, :], tps[:])
                        else:
                            nc.scalar.copy(aggT2[:, fc, :], tps[:])
                    # h2T[fout, n] = sum_fc W2[fc, fout]^T agg[fc, n]
                    h2t_ps = psm.tile([P, 2, P], F32, tag="mm")
                    for oc in range(2):
                        nc.tensor.matmul(
                            h2t_ps[:, oc, :], b2r_sb[:, oc, :], ones_row_sb[:],
                            start=True, stop=False,
                        )
                        for fc in range(2):
                            nc.tensor.matmul(
                                h2t_ps[:, oc, :],
                                w2_sb[:, fc, oc, :],
                                aggT2[:, fc, :],
                                start=False, stop=(fc == 1),
                            )
                    o2t = wk.tile([P, 2, P], BF16, tag="o2t")
                    nc.vector.tensor_scalar_max(o2t[:], h2t_ps[:], 0.0)
                    sp = psm.tile([P, K], F32, tag="mm")
                    for oc in range(2):
                        nc.tensor.matmul(
                            sp[:], o2t[:, oc, :], wp_sb[:, oc, :],
                            start=(oc == 0), stop=False,
                        )
                    nc.tensor.matmul(
                        sp[:], ones_row_sb[:], bp_sb[:], start=False, stop=True
                    )
                    smax = wk.tile([P, 1], F32, tag="smax")
                    nc.vector.tensor_reduce(
                        smax[:], sp[:], axis=mybir.AxisListType.X,
                        op=mybir.AluOpType.max, negate=True,
                    )
                    sexp = wk.tile([P, K], F32, tag="sexp")
                    ssum = wk.tile([P, 1], F32, tag="ssum")
                    nc.scalar.activation(
                        sexp[:], sp[:], mybir.ActivationFunctionType.Exp,
                        bias=smax[:], accum_out=ssum[:],
                    )
                    nc.vector.reciprocal(ssum[:], ssum[:])
                    nc.vector.tensor_scalar(
                        s_sb[:, b, :], sexp[:], ssum[:], mask_sb[:, b : b + 1],
                        op0=mybir.AluOpType.mult, op1=mybir.AluOpType.mult,
                    )
                    nc.scalar.activation(
                        sscratch[:], s_sb[:, b, :],
                        mybir.ActivationFunctionType.Square,
                        accum_out=ssq_sb[:, b : b + 1],
                    )
                    nc.vector.tensor_copy(spad_sb[:, b, 0:K], s_sb[:, b, :])
                nc.sync.dma_start(out=s_dr[:], in_=spad_sb[:].bitcast(F32))
                if _DEBUG_OUTPUTS:
                    nc.sync.dma_start(
                        out=dbg["s"][:].rearrange("(b p) k -> p b k", p=P),
                        in_=s_sb[:],
                    )

            # ---- ss/den/trss partials, packed into the RS2 payload
            if _MAX_PHASE >= 4:
                fin_ps = psa.tile([P, K + 8], F32, tag="fin")
                ss_ps = fin_ps[0:K, 0:K]
                smalls = fin_ps[:, K : K + 8]
                for b in range(BLK):
                    nc.tensor.matmul(
                        ss_ps, s_sb[:, b, :], s_sb[:, b, :],
                        start=(b == 0), stop=(b == BLK - 1),
                    )
                den_sb = wk.tile([P, BLK], F32, tag="den")
                nc.vector.tensor_tensor(
                    out=den_sb[:], in0=ssq_sb[:], in1=d_sb[:],
                    op=mybir.AluOpType.mult,
                )
                red2 = wk.tile([P, 1], F32, tag="red2")
                nc.vector.tensor_reduce(
                    red2[:], den_sb[:], axis=mybir.AxisListType.X,
                    op=mybir.AluOpType.add,
                )
                den_ps = smalls[0:1, 1:2]
                nc.tensor.matmul(den_ps, red2[:], ones_sb[:], start=True, stop=True)
                red3 = wk.tile([P, 1], F32, tag="red3")
                nc.vector.tensor_reduce(
                    red3[:], ssq_sb[:], axis=mybir.AxisListType.X,
                    op=mybir.AluOpType.add,
                )
                tr_ps = smalls[0:1, 2:3]
                nc.tensor.matmul(tr_ps, red3[:], ones_sb[:], start=True, stop=True)
                arbuf = wk.tile([K, K + 8], BF16, tag="arbuf")
                nc.vector.memset(arbuf[:], 0.0)
                nc.vector.tensor_copy(arbuf[:, 0:K], ss_ps)
                # hi/lo split of den/trss so the bf16 reduce stays accurate
                hi = wk.tile([1, 2], F32, tag="hi")
                nc.vector.tensor_copy(arbuf[0:1, K + 1 : K + 3], smalls[0:1, 1:3])
                nc.vector.tensor_copy(hi[:], arbuf[0:1, K + 1 : K + 3])
                lo = wk.tile([1, 2], F32, tag="lo")
                nc.vector.tensor_tensor(
                    out=lo[:], in0=smalls[0:1, 1:3], in1=hi[:],
                    op=mybir.AluOpType.subtract,
                )
                nc.vector.tensor_copy(arbuf[0:1, K + 5 : K + 7], lo[:])
                for c_ in range(C):
                    nc.sync.dma_start(
                        out=rs2_in[c_ * CH + SHARD : (c_ + 1) * CH, :], in_=arbuf[:]
                    )

            # ---------------- pool: col-sharded partial adj@s + RS

            for grp in range(NGRP if _MAX_PHASE >= 5 else 0):
                TS = SOFF3[grp + 1] - SOFF3[grp]
                msg = mp2.tile([P, TS, P // 2], F32, tag="msg3")
                nc.gpsimd.dma_gather(
                    msg[:], s_in[:],
                    pidx_sb[:, SOFF3[grp] * 8 : SOFF3[grp + 1] * 8],
                    TS * P, TS * P, P // 2, single_packet=False,
                )
                wts = wtp.tile([P, TS, P], FP8, tag="wts3")
                nc.sync.dma_start(
                    out=wts[:], in_=pwt_dr[:, SOFF3[grp] : SOFF3[grp + 1], :]
                )
                ev = evp.tile([P, GRP, K + 8], BF16, tag="ev3")
                nc.vector.memset(ev[:], 0.0)
                toff = 0
                for lb in range(GRP):
                    g = grp * GRP + lb
                    T = T3[g]
                    psum = ps.tile([P, K], F32, tag="scat")
                    for t2 in range(T // 2):
                        nc.tensor.matmul(
                            psum[:],
                            wts[:, toff + 2 * t2 : toff + 2 * t2 + 2, :],
                            msg[:, toff + 2 * t2 : toff + 2 * t2 + 2, :].bitcast(FP8)[:, :, 0:K],
                            start=(t2 == 0), stop=(t2 == T // 2 - 1),
                            perf_mode=mybir.MatmulPerfMode.DoubleRow,
                        )
                    toff += T
                    nc.vector.tensor_copy(ev[:, lb, 0:K], psum[:])
                ch = (grp * GRP) // BLK
                row0 = ch * CH + (grp * GRP - ch * BLK) * P
                nc.scalar.dma_start(
                    out=rs2_in[row0 : row0 + GRP * P, :].rearrange(
                        "(g p) k -> p g k", p=P
                    ),
                    in_=ev[:],
                )
            if _MAX_PHASE >= 6:
                RS(rs2_in[:], rs2_out[:])
                if _DEBUG_OUTPUTS:
                    nc.sync.dma_start(out=dbg["adjs"][:], in_=rs2_out[:])

            # ---------------- num partial + tiny AllGather + final scalar
            if _MAX_PHASE >= 7:
                adjsb = accp.tile([P, BLK, K + 8], BF16, tag="adjsb")
                nc.sync.dma_start(
                    out=adjsb[:],
                    in_=rs2_out[0:SHARD, :].rearrange("(b p) k -> p b k", p=P),
                )
                accb = wk.tile([K, K + 8], BF16, tag="accb")
                nc.sync.dma_start(out=accb[:], in_=rs2_out[SHARD:CH, :])
                acc = wk.tile([K, K + 8], F32, tag="acc")
                nc.vector.tensor_copy(acc[:], accb[:])
                # re-merge hi+lo scalars
                nc.vector.tensor_tensor(
                    out=acc[0:1, K + 1 : K + 3], in0=acc[0:1, K + 1 : K + 3],
                    in1=acc[0:1, K + 5 : K + 7], op=mybir.AluOpType.add,
                )
                nscratch = accp.tile([P, BLK, K], F32, tag="nscratch")
                nc.vector.tensor_tensor(
                    out=nscratch[:], in0=s_sb[:], in1=adjsb[:, :, 0:K],
                    op=mybir.AluOpType.mult,
                )
                red = wk.tile([P, 1], F32, tag="red")
                nc.vector.tensor_reduce(
                    red[:], nscratch[:].rearrange("p b k -> p (b k)"),
                    axis=mybir.AxisListType.X, op=mybir.AluOpType.add,
                )
                num_ps = smalls[0:1, 0:1]
                nc.tensor.matmul(num_ps, red[:], ones_sb[:], start=True, stop=True)
                numbuf = wk.tile([1, 1], F32, tag="numbuf")
                nc.vector.tensor_copy(numbuf[:], num_ps)
                nc.sync.dma_start(out=ar2_in[:], in_=numbuf[:])
                AG(ar2_in[:], ar2_out[:])
                # closed-form ortho from the RS2-reduced scalars (runs under AG)
                ss_tot = acc[:, 0:K]
                if _DEBUG_OUTPUTS:
                    nc.sync.dma_start(out=dbg["ss"][:], in_=ss_tot)
                sq64 = wk.tile([K, K], F32, tag="sq64")
                col64 = wk.tile([K, 1], F32, tag="col64")
                nc.scalar.activation(
                    sq64[:], ss_tot, mybir.ActivationFunctionType.Square,
                    accum_out=col64[:],
                )
                fro_ps = smalls[0:1, 3:4]
                nc.tensor.matmul(fro_ps, col64[:], ones_sb[:K, :], start=True, stop=True)
                rfro = wk.tile([1, 1], F32, tag="rfro")
                nc.scalar.sqrt(rfro[:], fro_ps)
                nc.vector.reciprocal(rfro[:], rfro[:])
                o2 = wk.tile([1, 1], F32, tag="o2s")
                nc.vector.tensor_tensor(
                    out=o2[:], in0=acc[0:1, K + 2 : K + 3], in1=rfro[:],
                    op=mybir.AluOpType.mult,
                )
                nc.vector.tensor_scalar(
                    o2[:], o2[:], -2.0 / float(np.sqrt(K)), 2.0,
                    op0=mybir.AluOpType.mult, op1=mybir.AluOpType.add,
                )
                orth = wk.tile([1, 1], F32, tag="orth_sb")
                nc.scalar.sqrt(orth[:], o2[:])
                rden = wk.tile([1, 1], F32, tag="rden")
                nc.vector.reciprocal(rden[:], acc[0:1, K + 1 : K + 2])
                g2 = wk.tile([1, C], F32, tag="g2")
                nc.sync.dma_start(out=g2[:], in_=ar2_out[:].rearrange("c f -> f c"))
                num_tot = wk.tile([1, 1], F32, tag="num_tot")
                nc.vector.tensor_reduce(
                    num_tot[:], g2[:], axis=mybir.AxisListType.X,
                    op=mybir.AluOpType.add,
                )
                if _DEBUG_OUTPUTS:
                    nd = wk.tile([1, 2], F32, tag="nd")
                    nc.vector.tensor_copy(nd[:, 0:1], num_tot[:])
                    nc.vector.tensor_copy(nd[:, 1:2], acc[0:1, K + 1 : K + 2])
                    nc.sync.dma_start(out=dbg["numden"][:], in_=nd[:])
                mloss = wk.tile([1, 1], F32, tag="mloss")
                nc.vector.tensor_tensor(
                    out=mloss[:], in0=num_tot[:], in1=rden[:],
                    op=mybir.AluOpType.mult,
                )
                res = wk.tile([1, 1], F32, tag="res")
                nc.vector.tensor_tensor(
                    out=res[:], in0=orth[:], in1=mloss[:],
                    op=mybir.AluOpType.subtract,
                )
                nc.sync.dma_start(out=out_t[:], in_=res[:])
            else:
                zz = wk.tile([1, 1], F32, tag="zz")
                nc.vector.memset(zz[:], 0.0)
                nc.sync.dma_start(out=out_t[:], in_=zz[:])

    if not for_sim:
        _split_excess_waits(nc)
    lower_extended_insts(nc)
    return nc


_PROG_CACHE = {}


def prog_key(tabs):
    return (tabs["T1"], tabs["T2"], tabs["T3"])


def _get_program(key, for_sim=False):
    k = (key, for_sim)
    if k not in _PROG_CACHE:
        _PROG_CACHE[k] = build_program(*key, for_sim=for_sim)
    return _PROG_CACHE[k]


def make_in_maps(inputs, tabs):
    x = np.asarray(inputs["x"], np.float32)
    W1, W2, Wp = (np.asarray(inputs[k], np.float32) for k in ("W1", "W2", "Wp"))
    b1, b2, bp = (np.asarray(inputs[k], np.float32) for k in ("b1", "b2", "bp"))
    xpad = np.zeros((NPAD, FIN), np.float32)
    xpad[:N] = x
    identb = np.eye(P, dtype=NPBF16)

    # W2 [256, 256] -> [fin_p, fc, oc, fout_p]: W2[fc*128+p, oc*128+q]
    w2r = np.ascontiguousarray(
        W2.reshape(2, P, 2, P).transpose(1, 0, 2, 3)
    ).astype(NPBF16)
    # Wp [256, 64] -> [fout_p, oc, k]
    wpr = np.ascontiguousarray(Wp.reshape(2, P, K).transpose(1, 0, 2)).astype(NPBF16)
    b2r = np.ascontiguousarray(b2.reshape(1, 2, P)).astype(np.float32)

    common = dict(
        W1=W1.astype(NPBF16),
        W2=w2r,
        Wp=wpr,
        b1=b1.reshape(1, FH).astype(np.float32),
        b2r=b2r,
        bp=bp.reshape(1, K).astype(np.float32),
        identb=identb,
        ones=np.ones((P, 1), np.float32),
        ones_row=np.ones((1, P), np.float32),
        x_full=np.ascontiguousarray(
            np.concatenate(
                [xpad.astype(ml_dtypes.float8_e4m3),
                 np.zeros((NPAD, FIN), ml_dtypes.float8_e4m3)], axis=1,
            )
        ).view(np.float32),
    )
    in_maps = []
    for c in range(C):
        in_maps.append(
            dict(
                common,
                l1_idx=tabs["l1_idx"][c], l1_wt=tabs["l1_wt"][c],
                l2_idx=tabs["l2_idx"][c], l2_wt=tabs["l2_wt"][c],
                p_idx=tabs["p_idx"][c], p_wt=tabs["p_wt"][c],
                d=tabs["d"][c],
                mask=tabs["mask"][c],
            )
        )
    return in_maps


def kernel(x, edge_index, edge_weight, W1, b1, W2, b2, Wp, bp):
    edge_index = np.asarray(edge_index)
    edge_weight = np.asarray(edge_weight, np.float32)
    tabs = preprocess(edge_index, edge_weight)
    nc = _get_program(prog_key(tabs))
    in_maps = make_in_maps(
        dict(x=x, W1=W1, b1=b1, W2=W2, b2=b2, Wp=Wp, bp=bp), tabs
    )
    trace = bool(int(os.environ.get("KERNEL_TRACE", "0")))
    kwargs = {}
    if trace:
        kwargs = dict(trace=True, tmpdir=os.environ.get("KERNEL_TRACE_DIR"))
    res = run_bass_kernel_spmd(nc, in_maps, core_ids=list(range(C)), **kwargs)
    if trace:
        kernel.exec_time_ns = res.exec_time_ns
        kernel.mean_exec_time_ns = res.mean_exec_time_ns
        kernel.bass_results = res
    out = res.results[0]["out"].reshape(())
    if _DEBUG_OUTPUTS:
        kernel.debug = {
            k: [res.results[c].get(f"dbg_{k}") for c in range(C)]
            for k in ("y1", "agg2", "s", "adjs", "numden", "ss")
        }
    return np.float32(out)


if __name__ == "__main__":
    import reference

    inputs = reference.setup_inputs()
    inputs = {k: np.asarray(v) for k, v in inputs.items()}
    got = kernel(**inputs)
    print("kernel out:", got)


# revision 61
# speedup vs baseline: 2.4989x; 1.0113x over previous
"""MinCutNet (2x GCN + dense_mincut_pool losses) as an 8-core Trainium2
Bass/Tile kernel.

Sharding / dataflow (v3):
  - L1 (aggregate x): dst-sharded. x is replicated (fp8, packed 4-per-f32
    so the SWDGE gather moves 64 elements/row), each core gathers edge
    sources for its own 1280 destination rows and scatter-accumulates via
    host-precomputed fp8 one-hot tiles with DoubleRow matmuls (256 edge
    slots per PE op). No collective.
  - L2 (aggregate y1): src-sharded. y1 is stored fp8 (256 feats = 64
    packed f32 gather elements). Each core computes PARTIAL aggregation
    sums for all 10240 destinations from its locally-owned sources, then
    one bf16 ReduceScatter sums partials and hands each core its rows.
    This replaces a 5.2MB y1 AllGather (~146us in the collective cost
    model) with a 655KB-out ReduceScatter (~31us).
  - dense + softmax: fully local per shard (W2/Wp in bf16, transposed
    dataflow so no activation transposes are needed before the matmuls).
  - pool (adj @ s): col-sharded partial sums like L2; the ss / den /
    tr(ss) scalar partials (hi/lo-split bf16 for accuracy) ride in 64
    extra rows of the same ReduceScatter payload, so the only remaining
    exchange afterwards is an 8-scalar AllGather for the mincut numerator.
  - ortho loss uses the closed form ||ss/|ss| - I/sqrt(K)||_F =
    sqrt(2 - 2 tr(ss) / (|ss|_F sqrt(K))).

One-hot scatter weight tiles (normalization folded in) are precomputed on
the host from edge_index/edge_weight only and streamed from HBM on the
otherwise-idle SP queue; gathers run on the Pool/SWDGE queue; evictions
and activations are spread across ACT and DVE so every phase is paced by
its gather-byte floor rather than a single engine.
"""

import os
import sys

sys.path.insert(0, "/opt/trn_rl_repo")

import numpy as np

import concourse.bass as bass
import concourse.mybir as mybir
import concourse.tile as tile
from concourse import library_config
from concourse.bass_utils import run_bass_kernel_spmd
from concourse.library_overlay import lower_extended_insts
from concourse.vector_clock import ScopedClock

import ml_dtypes

# ---------------------------------------------------------------- constants
N, E = 10000, 320000
FIN, FH, K = 128, 256, 64
C = 8               # cores
P = 128             # partitions
NPAD = 10240        # 80 blocks of 128
SHARD = NPAD // C   # 1280 nodes per core
BLK = SHARD // P    # 10 blocks per core
NBLK = NPAD // P    # 80 blocks total
F32 = mybir.dt.float32
BF16 = mybir.dt.bfloat16
FP8 = mybir.dt.float8e4
I16 = mybir.dt.int16
NPBF16 = ml_dtypes.bfloat16

_DEBUG_OUTPUTS = bool(int(os.environ.get("KERNEL_DEBUG_OUTPUTS", "0")))
_MAX_PHASE = int(os.environ.get("KERNEL_MAX_PHASE", "9"))


# ------------------------------------------------------- tile drain patch
def _patched_drain_and_barrier(self, tick_clock, wait_clock):
    """walrus in this container rejects >1 sync-wait command on the tail
    Drain; spread the waits across SP nops (1 wait each)."""
    nc = self.nc
    drain_inst = nc.sync.drain()
    wait_clock.add_sem_waits(
        drain_inst.ins, ScopedClock({None: tick_clock.global_clock})
    )
    waits = list(drain_inst.ins.sync_info.on_wait)
    if len(waits) > 1:
        upd = list(drain_inst.ins.sync_info.on_update)
        drain_inst.ins.sync_info = mybir.SyncInfo(on_wait=waits[:1], on_update=upd)
        for i, w in enumerate(waits[1:]):
            nop = nc.sync.nop(nofuse=True, hint=f"tailwait{i}")
            nop.ins.sync_info = mybir.SyncInfo(on_wait=[w], on_update=[])
    nc.all_engine_barrier()
    assert self.sems is not None
    popped = nc._tile_sem_poison_stack.pop()
    assert popped is self._sem_poison
    nc.clear_and_free_semaphores(list(self.sems.allocated().values()))
    nc.all_engine_barrier()


tile.TileContext._drain_and_barrier = _patched_drain_and_barrier

_noop_ctr = [0]


def _split_excess_waits(nc, lim=1):
    """walrus in this container caps sync-wait commands per instruction;
    spill excess waits onto same-engine NOPs placed just before."""
    nsplit = 0
    for fn in nc.m.functions:
        for b in fn.blocks:
            newl = []
            changed = False
            for inst in b.instructions:
                si = inst.sync_info
                if si is not None and len(si.on_wait) > lim:
                    waits = list(si.on_wait)
                    head, tail = waits[: len(waits) - lim], waits[len(waits) - lim :]
                    for i in range(0, len(head), lim):
                        _noop_ctr[0] += 1
                        nop = mybir.InstNoOp(
                            name=f"waitnop-{_noop_ctr[0]}",
                            sync_info=mybir.SyncInfo(
                                on_wait=head[i : i + lim], on_update=[]
                            ),
                            bass_nofuse=True,
                            engine=inst.engine,
                        )
                        newl.append(nop)
                    inst.sync_info = mybir.SyncInfo(
                        on_wait=tail, on_update=list(si.on_update)
                    )
                    nsplit += 1
                    changed = True
                newl.append(inst)
            if changed:
                b.instructions = newl
    return nsplit


# ------------------------------------------------------- host preprocessing
def _pack_idx(arr):
    """[NT*128] int -> dma_gather idx table [128, NT*8] int16.
    idx i lives at [i % 16, i // 16], replicated over 8 partition groups."""
    nt8 = arr.shape[0] // 16
    tab = arr.astype(np.int16).reshape(nt8, 16).T          # [16, NT*8]
    return np.ascontiguousarray(np.tile(tab, (8, 1)))      # [128, NT*8]


def _pack_val(arr):
    """[NT*128] f32 -> [128, NT] tile-major: [p, t] = arr[t*128 + p]."""
    nt = arr.shape[0] // P
    return np.ascontiguousarray(arr.reshape(nt, P).T)


def _bucketize(src, dst, w, owner, nbkt, bkt_of_dst, even=False):
    """Partition edges into per-(core, bucket) slot arrays.

    owner[e]     : core that processes edge e
    bkt_of_dst[e]: bucket (0..nbkt-1) within the core's loop
    Returns (T[nbkt] shared tile counts, per-core [C, NT*128] idx/dloc/w).
    """
    cnt = np.zeros((C, nbkt), np.int64)
    np.add.at(cnt, (owner, bkt_of_dst), 1)
    T = np.maximum(1, np.ceil(cnt.max(axis=0) / P).astype(np.int64))
    if even:
        T = ((T + 1) // 2) * 2
    off = np.concatenate([[0], np.cumsum(T)])
    NT = int(off[-1])
    a_idx = np.zeros((C, NT * P), np.int64)
    a_dloc = np.zeros((C, NT * P), np.float32)
    a_w = np.zeros((C, NT * P), np.float32)
    order = np.lexsort((bkt_of_dst, owner))
    src_s, dst_s, w_s = src[order], dst[order], w[order]
    own_s, bkt_s = owner[order], bkt_of_dst[order]
    # slot position within bucket
    lin = own_s * nbkt + bkt_s
    starts = np.searchsorted(lin, np.arange(C * nbkt))
    pos = np.arange(len(lin)) - starts[lin]
    slot = off[bkt_s] * P + pos
    a_idx[own_s, slot] = src_s
    a_dloc[own_s, slot] = (dst_s % P).astype(np.float32)
    a_w[own_s, slot] = w_s
    return T, a_idx, a_dloc, a_w


def _wt_table(dloc, w, npdt):
    """[C, NT*128] dloc/w -> dense one-hot scatter tiles [C, P, NT*P]:
    tab[c][p, t*128 + d] = (dloc[c, t*128+p] == d) * w[c, t*128+p]."""
    NT = dloc.shape[1] // P
    out = np.zeros((C, P, NT * P), npdt)
    rows = np.arange(NT * P)
    for c in range(C):
        wt_c = np.zeros((NT * P, P), np.float32)
        wt_c[rows, dloc[c].astype(np.int64)] = w[c]
        out[c] = np.ascontiguousarray(
            wt_c.reshape(NT, P, P).transpose(1, 0, 2).reshape(P, NT * P)
        ).astype(npdt)
    return out


def preprocess(edge_index, edge_weight):
    row = edge_index[0].astype(np.int64)
    col = edge_index[1].astype(np.int64)
    ew = edge_weight.astype(np.float32)

    # gcn_norm on host (edge data only): deg over col with self loops
    deg = np.zeros(NPAD, np.float32)
    np.add.at(deg, col, ew)
    deg[:N] += 1.0
    deg[deg == 0] = 1.0
    dis = (1.0 / np.sqrt(deg)).astype(np.float32)

    loops = np.arange(N, dtype=np.int64)
    gsrc = np.concatenate([row, loops])
    gdst = np.concatenate([col, loops])
    gnorm = np.concatenate([dis[row] * ew * dis[col], dis[:N] ** 2]).astype(np.float32)

    # L1: dst-sharded; bucket = local dst block (0..BLK-1)
    T1, l1_idx, l1_dloc, l1_w = _bucketize(
        gsrc, gdst, gnorm, owner=gdst // SHARD, nbkt=BLK,
        bkt_of_dst=(gdst % SHARD) // P, even=True,
    )
    # L2: src-sharded; bucket = global dst block (0..NBLK-1); idx local
    T2, l2_idx, l2_dloc, l2_w = _bucketize(
        gsrc % SHARD, gdst, gnorm, owner=gsrc // SHARD, nbkt=NBLK,
        bkt_of_dst=gdst // P, even=True,
    )
    l2_wt = _wt_table(l2_dloc, l2_w, ml_dtypes.float8_e4m3)

    # pool: col-sharded; gather s[col], scatter to row buckets, w = ew
    T3, p_idx, p_dloc, p_w = _bucketize(
        col % SHARD, row, ew, owner=col // SHARD, nbkt=NBLK,
        bkt_of_dst=row // P, even=True,
    )
    l1_wt = _wt_table(l1_dloc, l1_w, ml_dtypes.float8_e4m3)
    p_wt = _wt_table(p_dloc, p_w, ml_dtypes.float8_e4m3)

    # d = row degree sums (adj.sum(-1)); per-core [128, BLK]
    d = np.zeros(NPAD, np.float32)
    np.add.at(d, row, ew)
    d_sh = d.reshape(C, BLK, P).transpose(0, 2, 1)          # [C, 128, BLK]

    mask = np.zeros(NPAD, np.float32)
    mask[:N] = 1.0
    mask_sh = mask.reshape(C, BLK, P).transpose(0, 2, 1)

    tabs = dict(
        T1=tuple(int(t) for t in T1),
        T2=tuple(int(t) for t in T2),
        T3=tuple(int(t) for t in T3),
        l1_idx=np.stack([_pack_idx(a) for a in l1_idx]),
        l1_wt=l1_wt,
        l2_idx=np.stack([_pack_idx(a) for a in l2_idx]),
        l2_wt=l2_wt,
        p_idx=np.stack([_pack_idx(a) for a in p_idx]),
        p_wt=p_wt,
        d=np.ascontiguousarray(d_sh),
        mask=np.ascontiguousarray(mask_sh),
    )
    return tabs


# --------------------------------------------------------- device program
def build_program(T1, T2, T3, for_sim=False):
    NT1, NT2, NT3 = sum(T1), sum(T2), sum(T3)
    # gather-group size: GRP buckets share one dma_gather call
    GRP = 5
    NGRP = NBLK // GRP
    GRP1 = 1
    NGRP1 = BLK // GRP1
    off1 = np.concatenate([[0], np.cumsum(T1)])
    off2 = np.concatenate([[0], np.cumsum(T2)])
    off3 = np.concatenate([[0], np.cumsum(T3)])
    SOFF1 = [int(off1[g * GRP1]) for g in range(NGRP1 + 1)]
    SOFF2 = [int(off2[g * GRP]) for g in range(NGRP + 1)]
    SOFF3 = [int(off3[g * GRP]) for g in range(NGRP + 1)]

    nc = bass.Bass(num_devices=C)
    dp = nc.declare_dram_parameter

    x_fl = dp("x_full", [NPAD, FIN // 2], F32, isOutput=False)  # 128 bf16 packed
    w1 = dp("W1", [FIN, FH], BF16, isOutput=False)           # [128, 256]
    w2 = dp("W2", [P, 2, 2, P], BF16, isOutput=False)        # [fin_p, fc, oc, fout_p]
    wp = dp("Wp", [P, 2, K], BF16, isOutput=False)           # [fout_p, oc, k]
    b1 = dp("b1", [1, FH], F32, isOutput=False)
    b2r = dp("b2r", [1, 2, P], F32, isOutput=False)          # [1, oc, fout]
    bp = dp("bp", [1, K], F32, isOutput=False)
    l1_idx = dp("l1_idx", [P, NT1 * 8], I16, isOutput=False)
    l1_wt = dp("l1_wt", [P, NT1 * P], FP8, isOutput=False)
    l2_idx = dp("l2_idx", [P, NT2 * 8], I16, isOutput=False)
    l2_wt = dp("l2_wt", [P, NT2 * P], FP8, isOutput=False)
    p_idx = dp("p_idx", [P, NT3 * 8], I16, isOutput=False)
    p_wt = dp("p_wt", [P, NT3 * P], FP8, isOutput=False)
    d_t = dp("d", [P, BLK], F32, isOutput=False)
    mask_t = dp("mask", [P, BLK], F32, isOutput=False)
    identb_t = dp("identb", [P, P], BF16, isOutput=False)
    ones_t = dp("ones", [P, 1], F32, isOutput=False)
    ones_row_t = dp("ones_row", [1, P], F32, isOutput=False)

    out_t = dp("out", [1, 1], F32, isOutput=True)
    dbg = {}
    if _DEBUG_OUTPUTS:
        dbg["y1"] = dp("dbg_y1", [SHARD, FH], FP8, isOutput=True)
        dbg["agg2"] = dp("dbg_agg2", [SHARD, FH], BF16, isOutput=True)
        dbg["s"] = dp("dbg_s", [SHARD, K], F32, isOutput=True)
        dbg["adjs"] = dp("dbg_adjs", [SHARD, K], BF16, isOutput=True)
        dbg["numden"] = dp("dbg_numden", [1, 2], F32, isOutput=True)
        dbg["ss"] = dp("dbg_ss", [K, K], F32, isOutput=True)

    # internal DRAM
    y1_in = nc.dram_tensor("y1_in", [SHARD, FH // 4], F32)  # 256 fp8 packed
    s_in = nc.dram_tensor("s_in", [SHARD, P // 2], F32)  # 256 fp8 packed
    rs1_in = nc.dram_tensor("rs1_in", [NPAD, FH], BF16)
    rs1_out = nc.dram_tensor("rs1_out", [SHARD, FH], BF16)
    CH = SHARD + K  # rows per chunk: adjs shard + packed ss/den/trss
    rs2_in = nc.dram_tensor("rs2_in", [C * CH, K + 8], BF16)
    rs2_out = nc.dram_tensor("rs2_out", [CH, K + 8], BF16)
    ar_in = nc.dram_tensor("ar_in", [K, K + 2], F32)
    ar_out = nc.dram_tensor("ar_out", [C * K, K + 2], F32, addr_space="Shared")
    ar2_in = nc.dram_tensor("ar2_in", [1, 1], F32)
    ar2_out = nc.dram_tensor("ar2_out", [C, 1], F32, addr_space="Shared")

    rg = [list(range(C))]

    def RS(i, o):
        return nc.gpsimd.collective_compute(
            "ReduceScatter", mybir.AluOpType.add, replica_groups=rg, ins=[i], outs=[o]
        )

    def AG(i, o):
        return nc.gpsimd.collective_compute(
            "AllGather", mybir.AluOpType.bypass, replica_groups=rg, ins=[i], outs=[o]
        )

    nc.gpsimd.load_library(library_config.mlp)

    with tile.TileContext(nc) as tc:
        with (
            tc.tile_pool(name="const", bufs=1) as cp,
            tc.tile_pool(name="tabs", bufs=1) as tp,
            tc.tile_pool(name="msg", bufs=3) as mp,
            tc.tile_pool(name="msg2", bufs=4) as mp2,
            tc.tile_pool(name="wt", bufs=3) as wtp,
            tc.tile_pool(name="work", bufs=3) as wk,
            tc.tile_pool(name="ev", bufs=4) as evp,
            tc.tile_pool(name="acc", bufs=1) as accp,
            tc.tile_pool(name="ps", bufs=3, space="PSUM") as ps,
            tc.tile_pool(name="psm", bufs=2, space="PSUM") as psm,
            tc.tile_pool(name="pst", bufs=2, space="PSUM") as pst,
            tc.tile_pool(name="psa", bufs=1, space="PSUM") as psa,
        ):
            # ---------------- constants / tables into SBUF
            def load(pool, name, src, shape, dtype=F32, eng=None):
                t = pool.tile(shape, dtype, tag=name)
                (eng or nc.scalar).dma_start(out=t[:], in_=src)
                return t

            identb_sb = load(cp, "identb", identb_t[:], [P, P], BF16)
            ones_sb = load(cp, "ones", ones_t[:], [P, 1])
            ones_row_sb = load(cp, "ones_row", ones_row_t[:], [1, P])
            w1_sb = load(cp, "w1", w1[:], [P, FH], BF16)
            w2_sb = load(cp, "w2", w2[:], [P, 2, 2, P], BF16)
            wp_sb = load(cp, "wp", wp[:], [P, 2, K], BF16)
            b1_sb = load(cp, "b1", b1[:], [1, FH])
            b2r_sb = load(cp, "b2r", b2r[:], [1, 2, P])
            bp_sb = load(cp, "bp", bp[:], [1, K])
            d_sb = load(cp, "d", d_t[:], [P, BLK])
            mask_sb = load(cp, "mask", mask_t[:], [P, BLK])

            l1idx_sb = load(tp, "l1idx", l1_idx[:], [P, NT1 * 8], I16, eng=nc.scalar)
            l2idx_sb = load(tp, "l2idx", l2_idx[:], [P, NT2 * 8], I16)
            pidx_sb = load(tp, "pidx", p_idx[:], [P, NT3 * 8], I16)

            l1wt_dr = l1_wt[:].rearrange("p (t d) -> p t d", d=P)
            pwt_dr = p_wt[:].rearrange("p (t d) -> p t d", d=P)

            # ---------------- L1: dst-sharded aggregate of x, then W1+relu
            y1_sb = accp.tile([P, BLK, FH], FP8, tag="y1")
            y1_dr = y1_in[:].rearrange("(b p) f -> p b f", p=P)
            for grp in range(NGRP1 if _MAX_PHASE >= 1 else 0):
                TS = SOFF1[grp + 1] - SOFF1[grp]
                msg = mp.tile([P, TS, FIN // 2], F32, tag="msg1")
                nc.gpsimd.dma_gather(
                    msg[:], x_fl[:],
                    l1idx_sb[:, SOFF1[grp] * 8 : SOFF1[grp + 1] * 8],
                    TS * P, TS * P, FIN // 2, single_packet=False,
                )
                wts = wtp.tile([P, TS, P], FP8, tag="wts1")
                nc.sync.dma_start(
                    out=wts[:], in_=l1wt_dr[:, SOFF1[grp] : SOFF1[grp + 1], :]
                )
                toff = 0
                for lb in range(GRP1):
                    b = grp * GRP1 + lb
                    T = T1[b]
                    psum = ps.tile([P, FIN], F32, tag="scat")
                    for t2 in range(T // 2):
                        nc.tensor.matmul(
                            psum[:],
                            wts[:, toff + 2 * t2 : toff + 2 * t2 + 2, :],
                            msg[:, toff + 2 * t2 : toff + 2 * t2 + 2, :].bitcast(FP8)[:, :, 0:FIN],
                            start=(t2 == 0), stop=(t2 == T // 2 - 1),
                            perf_mode=mybir.MatmulPerfMode.DoubleRow,
                        )
                    toff += T
                    # dense: y1 = relu(agg @ W1 + b1)
                    aggb = wk.tile([P, FIN], BF16, tag="aggb1")
                    nc.scalar.copy(aggb[:], psum[:])
                    aggT_ps = pst.tile([P, P], BF16, tag="tr")
                    nc.tensor.transpose(aggT_ps[:], aggb[:], identb_sb[:])
                    aggT = wk.tile([P, P], BF16, tag="aggT1s")
                    nc.vector.tensor_copy(aggT[:], aggT_ps[:])
                    h_ps = psm.tile([P, FH], F32, tag="mm")
                    nc.tensor.matmul(h_ps[:], aggT[:], w1_sb[:], start=True, stop=False)
                    nc.tensor.matmul(
                        h_ps[:], ones_row_sb[:], b1_sb[:], start=False, stop=True
                    )
                    nc.scalar.activation(
                        y1_sb[:, b, :], h_ps[:], mybir.ActivationFunctionType.Relu
                    )
                    nc.scalar.dma_start(out=y1_dr[:, b, :], in_=y1_sb[:, b, :].bitcast(F32))
            if _DEBUG_OUTPUTS and _MAX_PHASE >= 1:
                nc.sync.dma_start(out=dbg["y1"][:], in_=y1_in[:])

            # ---------------- L2: src-sharded partial aggregation + RS
            # one-hot wt tiles are host-precomputed fp8 and streamed from HBM
            rs1_dr = rs1_in[:].rearrange("(g p) f -> p g f", p=P)
            l2wt_dr = l2_wt[:].rearrange("p (t d) -> p t d", d=P)
            for grp in range(NGRP if _MAX_PHASE >= 2 else 0):
                TS = SOFF2[grp + 1] - SOFF2[grp]
                msg = mp2.tile([P, TS, FH // 4], F32, tag="msg2")
                nc.gpsimd.dma_gather(
                    msg[:], y1_in[:],
                    l2idx_sb[:, SOFF2[grp] * 8 : SOFF2[grp + 1] * 8],
                    TS * P, TS * P, FH // 4, single_packet=False,
                )
                wts = wtp.tile([P, TS, P], FP8, tag="wts2")
                nc.sync.dma_start(
                    out=wts[:], in_=l2wt_dr[:, SOFF2[grp] : SOFF2[grp + 1], :]
                )
                ev = evp.tile([P, GRP, FH], BF16, tag="evT")
                toff = 0
                for lb in range(GRP):
                    g = grp * GRP + lb
                    T = T2[g]
                    psum = ps.tile([P, FH], F32, tag="scat")
                    for t2 in range(T // 2):
                        # fp8 DoubleRow: 256 edge slots per matmul
                        nc.tensor.matmul(
                            psum[:],
                            wts[:, toff + 2 * t2 : toff + 2 * t2 + 2, :],
                            msg[:, toff + 2 * t2 : toff + 2 * t2 + 2, :].bitcast(FP8),
                            start=(t2 == 0), stop=(t2 == T // 2 - 1),
                            perf_mode=mybir.MatmulPerfMode.DoubleRow,
                        )
                    toff += T
                    if lb == 3:
                        nc.scalar.copy(ev[:, lb, :], psum[:])
                    else:
                        nc.vector.tensor_copy(ev[:, lb, :], psum[:])
                nc.scalar.dma_start(
                    out=rs1_dr[:, grp * GRP : (grp + 1) * GRP, :], in_=ev[:]
                )
            if _MAX_PHASE >= 3:
                RS(rs1_in[:], rs1_out[:])
                if _DEBUG_OUTPUTS:
                    nc.sync.dma_start(out=dbg["agg2"][:], in_=rs1_out[:])

            # ---------------- local dense: y2 = relu(agg@W2+b2); s = softmax
            s_sb = accp.tile([P, BLK, K], F32, tag="s")
            spad_sb = accp.tile([P, BLK, FH], FP8, tag="spad")
            ssq_sb = accp.tile([P, BLK], F32, tag="ssq")
            sscratch = wk.tile([P, K], F32, tag="sscratch")
            s_dr = s_in[:].rearrange("(b p) k -> p b k", p=P)
            if _MAX_PHASE >= 4:
                nc.vector.memset(spad_sb[:], 0.0)
                aggsb = accp.tile([P, BLK, FH], BF16, tag="aggsb")
                rs1o_dr = rs1_out[:].rearrange("(b p) f -> p b f", p=P)
                nc.sync.dma_start(out=aggsb[:, 0:2, :], in_=rs1o_dr[:, 0:2, :])
                nc.sync.dma_start(out=aggsb[:, 2:BLK, :], in_=rs1o_dr[:, 2:BLK, :])
                for b in range(BLK):
                    aggT2 = wk.tile([P, 2, P], BF16, tag="aggT2")
                    for fc in range(2):
                        tps = pst.tile([P, P], BF16, tag="tr")
                        nc.tensor.transpose(
                            tps[:], aggsb[:, b, fc * P : (fc + 1) * P], identb_sb[:]
                        )
                        if fc == 0:
                            nc.vector.tensor_copy(aggT2[:, fc, :], tps[:])
                        else:
                            nc.scalar.copy(aggT2[:, fc, :], tps[:])
                    # h2T[fout, n] = sum_fc W2[fc, fout]^T agg[fc, n]
                    h2t_ps = psm.tile([P, 2, P], F32, tag="mm")
                    for oc in range(2):
                        nc.tensor.matmul(
                            h2t_ps[:, oc, :], b2r_sb[:, oc, :], ones_row_sb[:],
                            start=True, stop=False,
                        )
                        for fc in range(2):
                            nc.tensor.matmul(
                                h2t_ps[:, oc, :],
                                w2_sb[:, fc, oc, :],
                                aggT2[:, fc, :],
                                start=False, stop=(fc == 1),
                            )
                    o2t = wk.tile([P, 2, P], BF16, tag="o2t")
                    nc.vector.tensor_scalar_max(o2t[:], h2t_ps[:], 0.0)
                    sp = psm.tile([P, K], F32, tag="mm")
                    for oc in range(2):
                        nc.tensor.matmul(
                            sp[:], o2t[:, oc, :], wp_sb[:, oc, :],
                            start=(oc == 0), stop=False,
                        )
                    nc.tensor.matmul(
                        sp[:], ones_row_sb[:], bp_sb[:], start=False, stop=True
                    )
                    smax = wk.tile([P, 1], F32, tag="smax")
                    nc.vector.tensor_reduce(
                        smax[:], sp[:], axis=mybir.AxisListType.X,
                        op=mybir.AluOpType.max, negate=True,
                    )
                    sexp = wk.tile([P, K], F32, tag="sexp")
                    ssum = wk.tile([P, 1], F32, tag="ssum")
                    nc.scalar.activation(
                        sexp[:], sp[:], mybir.ActivationFunctionType.Exp,
                        bias=smax[:], accum_out=ssum[:],
                    )
                    nc.vector.reciprocal(ssum[:], ssum[:])
                    nc.vector.tensor_scalar(
                        s_sb[:, b, :], sexp[:], ssum[:], mask_sb[:, b : b + 1],
                        op0=mybir.AluOpType.mult, op1=mybir.AluOpType.mult,
                    )
                    nc.scalar.activation(
                        sscratch[:], s_sb[:, b, :],
                        mybir.ActivationFunctionType.Square,
                        accum_out=ssq_sb[:, b : b + 1],
                    )
                    nc.vector.tensor_copy(spad_sb[:, b, 0:K], s_sb[:, b, :])
                    nc.sync.dma_start(
                        out=s_dr[:, b, :], in_=spad_sb[:, b, :].bitcast(F32)
                    )
                if _DEBUG_OUTPUTS:
                    nc.sync.dma_start(
                        out=dbg["s"][:].rearrange("(b p) k -> p b k", p=P),
                        in_=s_sb[:],
                    )

            # ---- ss/den/trss partials, packed into the RS2 payload
            if _MAX_PHASE >= 4:
                fin_ps = psa.tile([P, K + 8], F32, tag="fin")
                ss_ps = fin_ps[0:K, 0:K]
                smalls = fin_ps[:, K : K + 8]
                for b in range(BLK):
                    nc.tensor.matmul(
                        ss_ps, s_sb[:, b, :], s_sb[:, b, :],
                        start=(b == 0), stop=(b == BLK - 1),
                    )
                den_sb = wk.tile([P, BLK], F32, tag="den")
                nc.vector.tensor_tensor(
                    out=den_sb[:], in0=ssq_sb[:], in1=d_sb[:],
                    op=mybir.AluOpType.mult,
                )
                red2 = wk.tile([P, 1], F32, tag="red2")
                nc.vector.tensor_reduce(
                    red2[:], den_sb[:], axis=mybir.AxisListType.X,
                    op=mybir.AluOpType.add,
                )
                den_ps = smalls[0:1, 1:2]
                nc.tensor.matmul(den_ps, red2[:], ones_sb[:], start=True, stop=True)
                red3 = wk.tile([P, 1], F32, tag="red3")
                nc.vector.tensor_reduce(
                    red3[:], ssq_sb[:], axis=mybir.AxisListType.X,
                    op=mybir.AluOpType.add,
                )
                tr_ps = smalls[0:1, 2:3]
                nc.tensor.matmul(tr_ps, red3[:], ones_sb[:], start=True, stop=True)
                arbuf = wk.tile([K, K + 8], BF16, tag="arbuf")
                nc.vector.memset(arbuf[:], 0.0)
                nc.vector.tensor_copy(arbuf[:, 0:K], ss_ps)
                # hi/lo split of den/trss so the bf16 reduce stays accurate
                hi = wk.tile([1, 2], F32, tag="hi")
                nc.vector.tensor_copy(arbuf[0:1, K + 1 : K + 3], smalls[0:1, 1:3])
                nc.vector.tensor_copy(hi[:], arbuf[0:1, K + 1 : K + 3])
                lo = wk.tile([1, 2], F32, tag="lo")
                nc.vector.tensor_tensor(
                    out=lo[:], in0=smalls[0:1, 1:3], in1=hi[:],
                    op=mybir.AluOpType.subtract,
                )
                nc.vector.tensor_copy(arbuf[0:1, K + 5 : K + 7], lo[:])
                for c_ in range(C):
                    nc.sync.dma_start(
                        out=rs2_in[c_ * CH + SHARD : (c_ + 1) * CH, :], in_=arbuf[:]
                    )

            # ---------------- pool: col-sharded partial adj@s + RS

            for grp in range(NGRP if _MAX_PHASE >= 5 else 0):
                TS = SOFF3[grp + 1] - SOFF3[grp]
                msg = mp2.tile([P, TS, P // 2], F32, tag="msg3")
                nc.gpsimd.dma_gather(
                    msg[:], s_in[:],
                    pidx_sb[:, SOFF3[grp] * 8 : SOFF3[grp + 1] * 8],
                    TS * P, TS * P, P // 2, single_packet=False,
                )
                wts = wtp.tile([P, TS, P], FP8, tag="wts3")
                nc.sync.dma_start(
                    out=wts[:], in_=pwt_dr[:, SOFF3[grp] : SOFF3[grp + 1], :]
                )
                ev = evp.tile([P, GRP, K + 8], BF16, tag="ev3")
                nc.vector.memset(ev[:], 0.0)
                toff = 0
                for lb in range(GRP):
                    g = grp * GRP + lb
                    T = T3[g]
                    psum = ps.tile([P, K], F32, tag="scat")
                    for t2 in range(T // 2):
                        nc.tensor.matmul(
                            psum[:],
                            wts[:, toff + 2 * t2 : toff + 2 * t2 + 2, :],
                            msg[:, toff + 2 * t2 : toff + 2 * t2 + 2, :].bitcast(FP8)[:, :, 0:K],
                            start=(t2 == 0), stop=(t2 == T // 2 - 1),
                            perf_mode=mybir.MatmulPerfMode.DoubleRow,
                        )
                    toff += T
                    nc.vector.tensor_copy(ev[:, lb, 0:K], psum[:])
                ch = (grp * GRP) // BLK
                row0 = ch * CH + (grp * GRP - ch * BLK) * P
                nc.scalar.dma_start(
                    out=rs2_in[row0 : row0 + GRP * P, :].rearrange(
                        "(g p) k -> p g k", p=P
                    ),
                    in_=ev[:],
                )
            if _MAX_PHASE >= 6:
                RS(rs2_in[:], rs2_out[:])
                if _DEBUG_OUTPUTS:
                    nc.sync.dma_start(out=dbg["adjs"][:], in_=rs2_out[:])

            # ---------------- num partial + tiny AllGather + final scalar
            if _MAX_PHASE >= 7:
                adjsb = accp.tile([P, BLK, K + 8], BF16, tag="adjsb")
                rs2o_dr = rs2_out[0:SHARD, :].rearrange("(b p) k -> p b k", p=P)
                H = BLK // 2
                nc.sync.dma_start(out=adjsb[:, 0:H, :], in_=rs2o_dr[:, 0:H, :])
                nc.sync.dma_start(out=adjsb[:, H:BLK, :], in_=rs2o_dr[:, H:BLK, :])
                accb = wk.tile([K, K + 8], BF16, tag="accb")
                nc.sync.dma_start(out=accb[:], in_=rs2_out[SHARD:CH, :])
                acc = wk.tile([K, K + 8], F32, tag="acc")
                nc.vector.tensor_copy(acc[:], accb[:])
                # re-merge hi+lo scalars
                nc.vector.tensor_tensor(
                    out=acc[0:1, K + 1 : K + 3], in0=acc[0:1, K + 1 : K + 3],
                    in1=acc[0:1, K + 5 : K + 7], op=mybir.AluOpType.add,
                )
                nscratch = accp.tile([P, BLK, K], F32, tag="nscratch")
                nc.vector.tensor_tensor(
                    out=nscratch[:, 0:H, :], in0=s_sb[:, 0:H, :],
                    in1=adjsb[:, 0:H, 0:K], op=mybir.AluOpType.mult,
                )
                nc.vector.tensor_tensor(
                    out=nscratch[:, H:BLK, :], in0=s_sb[:, H:BLK, :],
                    in1=adjsb[:, H:BLK, 0:K], op=mybir.AluOpType.mult,
                )
                red = wk.tile([P, 1], F32, tag="red")
                nc.vector.tensor_reduce(
                    red[:], nscratch[:].rearrange("p b k -> p (b k)"),
                    axis=mybir.AxisListType.X, op=mybir.AluOpType.add,
                )
                num_ps = smalls[0:1, 0:1]
                nc.tensor.matmul(num_ps, red[:], ones_sb[:], start=True, stop=True)
                numbuf = wk.tile([1, 1], F32, tag="numbuf")
                nc.vector.tensor_copy(numbuf[:], num_ps)
                nc.sync.dma_start(out=ar2_in[:], in_=numbuf[:])
                AG(ar2_in[:], ar2_out[:])
                # closed-form ortho from the RS2-reduced scalars (runs under AG)
                ss_tot = acc[:, 0:K]
                if _DEBUG_OUTPUTS:
                    nc.sync.dma_start(out=dbg["ss"][:], in_=ss_tot)
                sq64 = wk.tile([K, K], F32, tag="sq64")
                col64 = wk.tile([K, 1], F32, tag="col64")
                nc.scalar.activation(
                    sq64[:], ss_tot, mybir.ActivationFunctionType.Square,
                    accum_out=col64[:],
                )
                fro_ps = smalls[0:1, 3:4]
                nc.tensor.matmul(fro_ps, col64[:], ones_sb[:K, :], start=True, stop=True)
                rfro = wk.tile([1, 1], F32, tag="rfro")
                nc.scalar.sqrt(rfro[:], fro_ps)
                nc.vector.reciprocal(rfro[:], rfro[:])
                o2 = wk.tile([1, 1], F32, tag="o2s")
                nc.vector.tensor_tensor(
                    out=o2[:], in0=acc[0:1, K + 2 : K + 3], in1=rfro[:],
                    op=mybir.AluOpType.mult,
                )
                nc.vector.tensor_scalar(
                    o2[:], o2[:], -2.0 / float(np.sqrt(K)), 2.0,
                    op0=mybir.AluOpType.mult, op1=mybir.AluOpType.add,
                )
                orth = wk.tile([1, 1], F32, tag="orth_sb")
                nc.scalar.sqrt(orth[:], o2[:])
                rden = wk.tile([1, 1], F32, tag="rden")
                nc.vector.reciprocal(rden[:], acc[0:1, K + 1 : K + 2])
                g2 = wk.tile([1, C], F32, tag="g2")
                nc.sync.dma_start(out=g2[:], in_=ar2_out[:].rearrange("c f -> f c"))
                num_tot = wk.tile([1, 1], F32, tag="num_tot")
                nc.vector.tensor_reduce(
                    num_tot[:], g2[:], axis=mybir.AxisListType.X,
                    op=mybir.AluOpType.add,
                )
                if _DEBUG_OUTPUTS:
                    nd = wk.tile([1, 2], F32, tag="nd")
                    nc.vector.tensor_copy(nd[:, 0:1], num_tot[:])
                    nc.vector.tensor_copy(nd[:, 1:2], acc[0:1, K + 1 : K + 2])
                    nc.sync.dma_start(out=dbg["numden"][:], in_=nd[:])
                mloss = wk.tile([1, 1], F32, tag="mloss")
                nc.vector.tensor_tensor(
                    out=mloss[:], in0=num_tot[:], in1=rden[:],
                    op=mybir.AluOpType.mult,
                )
                res = wk.tile([1, 1], F32, tag="res")
                nc.vector.tensor_tensor(
                    out=res[:], in0=orth[:], in1=mloss[:],
                    op=mybir.AluOpType.subtract,
                )
                nc.sync.dma_start(out=out_t[:], in_=res[:])
            else:
                zz = wk.tile([1, 1], F32, tag="zz")
                nc.vector.memset(zz[:], 0.0)
                nc.sync.dma_start(out=out_t[:], in_=zz[:])

    if not for_sim:
        _split_excess_waits(nc)
    lower_extended_insts(nc)
    return nc


_PROG_CACHE = {}


def prog_key(tabs):
    return (tabs["T1"], tabs["T2"], tabs["T3"])


def _get_program(key, for_sim=False):
    k = (key, for_sim)
    if k not in _PROG_CACHE:
        _PROG_CACHE[k] = build_program(*key, for_sim=for_sim)
    return _PROG_CACHE[k]


def make_in_maps(inputs, tabs):
    x = np.asarray(inputs["x"], np.float32)
    W1, W2, Wp = (np.asarray(inputs[k], np.float32) for k in ("W1", "W2", "Wp"))
    b1, b2, bp = (np.asarray(inputs[k], np.float32) for k in ("b1", "b2", "bp"))
    xpad = np.zeros((NPAD, FIN), np.float32)
    xpad[:N] = x
    identb = np.eye(P, dtype=NPBF16)

    # W2 [256, 256] -> [fin_p, fc, oc, fout_p]: W2[fc*128+p, oc*128+q]
    w2r = np.ascontiguousarray(
        W2.reshape(2, P, 2, P).transpose(1, 0, 2, 3)
    ).astype(NPBF16)
    # Wp [256, 64] -> [fout_p, oc, k]
    wpr = np.ascontiguousarray(Wp.reshape(2, P, K).transpose(1, 0, 2)).astype(NPBF16)
    b2r = np.ascontiguousarray(b2.reshape(1, 2, P)).astype(np.float32)

    common = dict(
        W1=W1.astype(NPBF16),
        W2=w2r,
        Wp=wpr,
        b1=b1.reshape(1, FH).astype(np.float32),
        b2r=b2r,
        bp=bp.reshape(1, K).astype(np.float32),
        identb=identb,
        ones=np.ones((P, 1), np.float32),
        ones_row=np.ones((1, P), np.float32),
        x_full=np.ascontiguousarray(
            np.concatenate(
                [xpad.astype(ml_dtypes.float8_e4m3),
                 np.zeros((NPAD, FIN), ml_dtypes.float8_e4m3)], axis=1,
            )
        ).view(np.float32),
    )
    in_maps = []
    for c in range(C):
        in_maps.append(
            dict(
                common,
                l1_idx=tabs["l1_idx"][c], l1_wt=tabs["l1_wt"][c],
                l2_idx=tabs["l2_idx"][c], l2_wt=tabs["l2_wt"][c],
                p_idx=tabs["p_idx"][c], p_wt=tabs["p_wt"][c],
                d=tabs["d"][c],
                mask=tabs["mask"][c],
            )
        )
    return in_maps


def kernel(x, edge_index, edge_weight, W1, b1, W2, b2, Wp, bp):
    edge_index = np.asarray(edge_index)
    edge_weight = np.asarray(edge_weight, np.float32)
    tabs = preprocess(edge_index, edge_weight)
    nc = _get_program(prog_key(tabs))
    in_maps = make_in_maps(
        dict(x=x, W1=W1, b1=b1, W2=W2, b2=b2, Wp=Wp, bp=bp), tabs
    )
    trace = bool(int(os.environ.get("KERNEL_TRACE", "0")))
    kwargs = {}
    if trace:
        kwargs = dict(trace=True, tmpdir=os.environ.get("KERNEL_TRACE_DIR"))
    res = run_bass_kernel_spmd(nc, in_maps, core_ids=list(range(C)), **kwargs)
    if trace:
        kernel.exec_time_ns = res.exec_time_ns
        kernel.mean_exec_time_ns = res.mean_exec_time_ns
        kernel.bass_results = res
    out = res.results[0]["out"].reshape(())
    if _DEBUG_OUTPUTS:
        kernel.debug = {
            k: [res.results[c].get(f"dbg_{k}") for c in range(C)]
            for k in ("y1", "agg2", "s", "adjs", "numden", "ss")
        }
    return np.float32(out)


if __name__ == "__main__":
    import reference

    inputs = reference.setup_inputs()
    inputs = {k: np.asarray(v) for k, v in inputs.items()}
    got = kernel(**inputs)
    print("kernel out:", got)


# revision 62
# speedup vs baseline: 2.5425x; 1.0174x over previous
"""MinCutNet (2x GCN + dense_mincut_pool losses) as an 8-core Trainium2
Bass/Tile kernel.

Sharding / dataflow (v3):
  - L1 (aggregate x): dst-sharded. x is replicated (fp8, packed 4-per-f32
    so the SWDGE gather moves 64 elements/row), each core gathers edge
    sources for its own 1280 destination rows and scatter-accumulates via
    host-precomputed fp8 one-hot tiles with DoubleRow matmuls (256 edge
    slots per PE op). No collective.
  - L2 (aggregate y1): src-sharded. y1 is stored fp8 (256 feats = 64
    packed f32 gather elements). Each core computes PARTIAL aggregation
    sums for all 10240 destinations from its locally-owned sources, then
    one bf16 ReduceScatter sums partials and hands each core its rows.
    This replaces a 5.2MB y1 AllGather (~146us in the collective cost
    model) with a 655KB-out ReduceScatter (~31us).
  - dense + softmax: fully local per shard (W2/Wp in bf16, transposed
    dataflow so no activation transposes are needed before the matmuls).
  - pool (adj @ s): col-sharded partial sums like L2; the ss / den /
    tr(ss) scalar partials (hi/lo-split bf16 for accuracy) ride in 64
    extra rows of the same ReduceScatter payload, so the only remaining
    exchange afterwards is an 8-scalar AllGather for the mincut numerator.
  - ortho loss uses the closed form ||ss/|ss| - I/sqrt(K)||_F =
    sqrt(2 - 2 tr(ss) / (|ss|_F sqrt(K))).

One-hot scatter weight tiles (normalization folded in) are precomputed on
the host from edge_index/edge_weight only and streamed from HBM on the
otherwise-idle SP queue; gathers run on the Pool/SWDGE queue; evictions
and activations are spread across ACT and DVE so every phase is paced by
its gather-byte floor rather than a single engine.
"""

import os
import sys

sys.path.insert(0, "/opt/trn_rl_repo")

import numpy as np

import concourse.bass as bass
import concourse.mybir as mybir
import concourse.tile as tile
from concourse import library_config
from concourse.bass_utils import run_bass_kernel_spmd
from concourse.library_overlay import lower_extended_insts
from concourse.vector_clock import ScopedClock

import ml_dtypes

# ---------------------------------------------------------------- constants
N, E = 10000, 320000
FIN, FH, K = 128, 256, 64
C = 8               # cores
P = 128             # partitions
NPAD = 10240        # 80 blocks of 128
SHARD = NPAD // C   # 1280 nodes per core
BLK = SHARD // P    # 10 blocks per core
NBLK = NPAD // P    # 80 blocks total
F32 = mybir.dt.float32
BF16 = mybir.dt.bfloat16
FP8 = mybir.dt.float8e4
I16 = mybir.dt.int16
NPBF16 = ml_dtypes.bfloat16

_DEBUG_OUTPUTS = bool(int(os.environ.get("KERNEL_DEBUG_OUTPUTS", "0")))
_MAX_PHASE = int(os.environ.get("KERNEL_MAX_PHASE", "9"))


# ------------------------------------------------------- tile drain patch
def _patched_drain_and_barrier(self, tick_clock, wait_clock):
    """walrus in this container rejects >1 sync-wait command on the tail
    Drain; spread the waits across SP nops (1 wait each)."""
    nc = self.nc
    drain_inst = nc.sync.drain()
    wait_clock.add_sem_waits(
        drain_inst.ins, ScopedClock({None: tick_clock.global_clock})
    )
    waits = list(drain_inst.ins.sync_info.on_wait)
    if len(waits) > 1:
        upd = list(drain_inst.ins.sync_info.on_update)
        drain_inst.ins.sync_info = mybir.SyncInfo(on_wait=waits[:1], on_update=upd)
        for i, w in enumerate(waits[1:]):
            nop = nc.sync.nop(nofuse=True, hint=f"tailwait{i}")
            nop.ins.sync_info = mybir.SyncInfo(on_wait=[w], on_update=[])
    nc.all_engine_barrier()
    assert self.sems is not None
    popped = nc._tile_sem_poison_stack.pop()
    assert popped is self._sem_poison
    nc.clear_and_free_semaphores(list(self.sems.allocated().values()))
    nc.all_engine_barrier()


tile.TileContext._drain_and_barrier = _patched_drain_and_barrier

_noop_ctr = [0]


def _split_excess_waits(nc, lim=1):
    """walrus in this container caps sync-wait commands per instruction;
    spill excess waits onto same-engine NOPs placed just before."""
    nsplit = 0
    for fn in nc.m.functions:
        for b in fn.blocks:
            newl = []
            changed = False
            for inst in b.instructions:
                si = inst.sync_info
                if si is not None and len(si.on_wait) > lim:
                    waits = list(si.on_wait)
                    head, tail = waits[: len(waits) - lim], waits[len(waits) - lim :]
                    for i in range(0, len(head), lim):
                        _noop_ctr[0] += 1
                        nop = mybir.InstNoOp(
                            name=f"waitnop-{_noop_ctr[0]}",
                            sync_info=mybir.SyncInfo(
                                on_wait=head[i : i + lim], on_update=[]
                            ),
                            bass_nofuse=True,
                            engine=inst.engine,
                        )
                        newl.append(nop)
                    inst.sync_info = mybir.SyncInfo(
                        on_wait=tail, on_update=list(si.on_update)
                    )
                    nsplit += 1
                    changed = True
                newl.append(inst)
            if changed:
                b.instructions = newl
    return nsplit


# ------------------------------------------------------- host preprocessing
def _pack_idx(arr):
    """[NT*128] int -> dma_gather idx table [128, NT*8] int16.
    idx i lives at [i % 16, i // 16], replicated over 8 partition groups."""
    nt8 = arr.shape[0] // 16
    tab = arr.astype(np.int16).reshape(nt8, 16).T          # [16, NT*8]
    return np.ascontiguousarray(np.tile(tab, (8, 1)))      # [128, NT*8]


def _pack_val(arr):
    """[NT*128] f32 -> [128, NT] tile-major: [p, t] = arr[t*128 + p]."""
    nt = arr.shape[0] // P
    return np.ascontiguousarray(arr.reshape(nt, P).T)


def _bucketize(src, dst, w, owner, nbkt, bkt_of_dst, even=False):
    """Partition edges into per-(core, bucket) slot arrays.

    owner[e]     : core that processes edge e
    bkt_of_dst[e]: bucket (0..nbkt-1) within the core's loop
    Returns (T[nbkt] shared tile counts, per-core [C, NT*128] idx/dloc/w).
    """
    cnt = np.zeros((C, nbkt), np.int64)
    np.add.at(cnt, (owner, bkt_of_dst), 1)
    T = np.maximum(1, np.ceil(cnt.max(axis=0) / P).astype(np.int64))
    if even:
        T = ((T + 1) // 2) * 2
    off = np.concatenate([[0], np.cumsum(T)])
    NT = int(off[-1])
    a_idx = np.zeros((C, NT * P), np.int64)
    a_dloc = np.zeros((C, NT * P), np.float32)
    a_w = np.zeros((C, NT * P), np.float32)
    order = np.lexsort((bkt_of_dst, owner))
    src_s, dst_s, w_s = src[order], dst[order], w[order]
    own_s, bkt_s = owner[order], bkt_of_dst[order]
    # slot position within bucket
    lin = own_s * nbkt + bkt_s
    starts = np.searchsorted(lin, np.arange(C * nbkt))
    pos = np.arange(len(lin)) - starts[lin]
    slot = off[bkt_s] * P + pos
    a_idx[own_s, slot] = src_s
    a_dloc[own_s, slot] = (dst_s % P).astype(np.float32)
    a_w[own_s, slot] = w_s
    return T, a_idx, a_dloc, a_w


def _wt_table(dloc, w, npdt):
    """[C, NT*128] dloc/w -> dense one-hot scatter tiles [C, P, NT*P]:
    tab[c][p, t*128 + d] = (dloc[c, t*128+p] == d) * w[c, t*128+p]."""
    NT = dloc.shape[1] // P
    out = np.zeros((C, P, NT * P), npdt)
    rows = np.arange(NT * P)
    for c in range(C):
        wt_c = np.zeros((NT * P, P), np.float32)
        wt_c[rows, dloc[c].astype(np.int64)] = w[c]
        out[c] = np.ascontiguousarray(
            wt_c.reshape(NT, P, P).transpose(1, 0, 2).reshape(P, NT * P)
        ).astype(npdt)
    return out


def preprocess(edge_index, edge_weight):
    row = edge_index[0].astype(np.int64)
    col = edge_index[1].astype(np.int64)
    ew = edge_weight.astype(np.float32)

    # gcn_norm on host (edge data only): deg over col with self loops
    deg = np.zeros(NPAD, np.float32)
    np.add.at(deg, col, ew)
    deg[:N] += 1.0
    deg[deg == 0] = 1.0
    dis = (1.0 / np.sqrt(deg)).astype(np.float32)

    loops = np.arange(N, dtype=np.int64)
    gsrc = np.concatenate([row, loops])
    gdst = np.concatenate([col, loops])
    gnorm = np.concatenate([dis[row] * ew * dis[col], dis[:N] ** 2]).astype(np.float32)

    # L1: dst-sharded; bucket = local dst block (0..BLK-1)
    T1, l1_idx, l1_dloc, l1_w = _bucketize(
        gsrc, gdst, gnorm, owner=gdst // SHARD, nbkt=BLK,
        bkt_of_dst=(gdst % SHARD) // P,
    )
    # L2: src-sharded; bucket = global dst block (0..NBLK-1); idx local
    T2, l2_idx, l2_dloc, l2_w = _bucketize(
        gsrc % SHARD, gdst, gnorm, owner=gsrc // SHARD, nbkt=NBLK,
        bkt_of_dst=gdst // P,
    )
    l2_wt = _wt_table(l2_dloc, l2_w, ml_dtypes.float8_e4m3)

    # pool: col-sharded; gather s[col], scatter to row buckets, w = ew
    T3, p_idx, p_dloc, p_w = _bucketize(
        col % SHARD, row, ew, owner=col // SHARD, nbkt=NBLK,
        bkt_of_dst=row // P,
    )
    l1_wt = _wt_table(l1_dloc, l1_w, ml_dtypes.float8_e4m3)
    p_wt = _wt_table(p_dloc, p_w, ml_dtypes.float8_e4m3)

    # d = row degree sums (adj.sum(-1)); per-core [128, BLK]
    d = np.zeros(NPAD, np.float32)
    np.add.at(d, row, ew)
    d_sh = d.reshape(C, BLK, P).transpose(0, 2, 1)          # [C, 128, BLK]

    mask = np.zeros(NPAD, np.float32)
    mask[:N] = 1.0
    mask_sh = mask.reshape(C, BLK, P).transpose(0, 2, 1)

    tabs = dict(
        T1=tuple(int(t) for t in T1),
        T2=tuple(int(t) for t in T2),
        T3=tuple(int(t) for t in T3),
        l1_idx=np.stack([_pack_idx(a) for a in l1_idx]),
        l1_wt=l1_wt,
        l2_idx=np.stack([_pack_idx(a) for a in l2_idx]),
        l2_wt=l2_wt,
        p_idx=np.stack([_pack_idx(a) for a in p_idx]),
        p_wt=p_wt,
        d=np.ascontiguousarray(d_sh),
        mask=np.ascontiguousarray(mask_sh),
    )
    return tabs


# --------------------------------------------------------- device program
def build_program(T1, T2, T3, for_sim=False):
    NT1, NT2, NT3 = sum(T1), sum(T2), sum(T3)
    # gather-group size: GRP buckets share one dma_gather call
    GRP = 5
    NGRP = NBLK // GRP
    GRP1 = 1
    NGRP1 = BLK // GRP1
    off1 = np.concatenate([[0], np.cumsum(T1)])
    off2 = np.concatenate([[0], np.cumsum(T2)])
    off3 = np.concatenate([[0], np.cumsum(T3)])
    SOFF1 = [int(off1[g * GRP1]) for g in range(NGRP1 + 1)]
    SOFF2 = [int(off2[g * GRP]) for g in range(NGRP + 1)]
    SOFF3 = [int(off3[g * GRP]) for g in range(NGRP + 1)]

    nc = bass.Bass(num_devices=C)
    dp = nc.declare_dram_parameter

    x_fl = dp("x_full", [NPAD, FIN // 2], F32, isOutput=False)  # 128 bf16 packed
    w1 = dp("W1", [FIN, FH], BF16, isOutput=False)           # [128, 256]
    w2 = dp("W2", [P, 2, 2, P], BF16, isOutput=False)        # [fin_p, fc, oc, fout_p]
    wp = dp("Wp", [P, 2, K], BF16, isOutput=False)           # [fout_p, oc, k]
    b1 = dp("b1", [1, FH], F32, isOutput=False)
    b2r = dp("b2r", [1, 2, P], F32, isOutput=False)          # [1, oc, fout]
    bp = dp("bp", [1, K], F32, isOutput=False)
    l1_idx = dp("l1_idx", [P, NT1 * 8], I16, isOutput=False)
    l1_wt = dp("l1_wt", [P, NT1 * P], FP8, isOutput=False)
    l2_idx = dp("l2_idx", [P, NT2 * 8], I16, isOutput=False)
    l2_wt = dp("l2_wt", [P, NT2 * P], FP8, isOutput=False)
    p_idx = dp("p_idx", [P, NT3 * 8], I16, isOutput=False)
    p_wt = dp("p_wt", [P, NT3 * P], FP8, isOutput=False)
    d_t = dp("d", [P, BLK], F32, isOutput=False)
    mask_t = dp("mask", [P, BLK], F32, isOutput=False)
    identb_t = dp("identb", [P, P], BF16, isOutput=False)
    ones_t = dp("ones", [P, 1], F32, isOutput=False)
    ones_row_t = dp("ones_row", [1, P], F32, isOutput=False)

    out_t = dp("out", [1, 1], F32, isOutput=True)
    dbg = {}
    if _DEBUG_OUTPUTS:
        dbg["y1"] = dp("dbg_y1", [SHARD, FH], FP8, isOutput=True)
        dbg["agg2"] = dp("dbg_agg2", [SHARD, FH], BF16, isOutput=True)
        dbg["s"] = dp("dbg_s", [SHARD, K], F32, isOutput=True)
        dbg["adjs"] = dp("dbg_adjs", [SHARD, K], BF16, isOutput=True)
        dbg["numden"] = dp("dbg_numden", [1, 2], F32, isOutput=True)
        dbg["ss"] = dp("dbg_ss", [K, K], F32, isOutput=True)

    # internal DRAM
    y1_in = nc.dram_tensor("y1_in", [SHARD, FH // 4], F32)  # 256 fp8 packed
    s_in = nc.dram_tensor("s_in", [SHARD, P // 2], F32)  # 256 fp8 packed
    rs1_in = nc.dram_tensor("rs1_in", [NPAD, FH], BF16)
    rs1_out = nc.dram_tensor("rs1_out", [SHARD, FH], BF16)
    CH = SHARD + K  # rows per chunk: adjs shard + packed ss/den/trss
    rs2_in = nc.dram_tensor("rs2_in", [C * CH, K + 8], BF16)
    rs2_out = nc.dram_tensor("rs2_out", [CH, K + 8], BF16)
    ar_in = nc.dram_tensor("ar_in", [K, K + 2], F32)
    ar_out = nc.dram_tensor("ar_out", [C * K, K + 2], F32, addr_space="Shared")
    ar2_in = nc.dram_tensor("ar2_in", [1, 1], F32)
    ar2_out = nc.dram_tensor("ar2_out", [C, 1], F32, addr_space="Shared")

    rg = [list(range(C))]

    def RS(i, o):
        return nc.gpsimd.collective_compute(
            "ReduceScatter", mybir.AluOpType.add, replica_groups=rg, ins=[i], outs=[o]
        )

    def AG(i, o):
        return nc.gpsimd.collective_compute(
            "AllGather", mybir.AluOpType.bypass, replica_groups=rg, ins=[i], outs=[o]
        )

    nc.gpsimd.load_library(library_config.mlp)

    with tile.TileContext(nc) as tc:
        with (
            tc.tile_pool(name="const", bufs=1) as cp,
            tc.tile_pool(name="tabs", bufs=1) as tp,
            tc.tile_pool(name="msg", bufs=3) as mp,
            tc.tile_pool(name="msg2", bufs=4) as mp2,
            tc.tile_pool(name="wt", bufs=3) as wtp,
            tc.tile_pool(name="work", bufs=3) as wk,
            tc.tile_pool(name="ev", bufs=4) as evp,
            tc.tile_pool(name="acc", bufs=1) as accp,
            tc.tile_pool(name="ps", bufs=3, space="PSUM") as ps,
            tc.tile_pool(name="psm", bufs=2, space="PSUM") as psm,
            tc.tile_pool(name="pst", bufs=2, space="PSUM") as pst,
            tc.tile_pool(name="psa", bufs=1, space="PSUM") as psa,
        ):
            # ---------------- constants / tables into SBUF
            def load(pool, name, src, shape, dtype=F32, eng=None):
                t = pool.tile(shape, dtype, tag=name)
                (eng or nc.scalar).dma_start(out=t[:], in_=src)
                return t

            identb_sb = load(cp, "identb", identb_t[:], [P, P], BF16)
            ones_sb = load(cp, "ones", ones_t[:], [P, 1])
            ones_row_sb = load(cp, "ones_row", ones_row_t[:], [1, P])
            w1_sb = load(cp, "w1", w1[:], [P, FH], BF16)
            w2_sb = load(cp, "w2", w2[:], [P, 2, 2, P], BF16)
            wp_sb = load(cp, "wp", wp[:], [P, 2, K], BF16)
            b1_sb = load(cp, "b1", b1[:], [1, FH])
            b2r_sb = load(cp, "b2r", b2r[:], [1, 2, P])
            bp_sb = load(cp, "bp", bp[:], [1, K])
            d_sb = load(cp, "d", d_t[:], [P, BLK])
            mask_sb = load(cp, "mask", mask_t[:], [P, BLK])

            l1idx_sb = load(tp, "l1idx", l1_idx[:], [P, NT1 * 8], I16, eng=nc.scalar)
            l2idx_sb = load(tp, "l2idx", l2_idx[:], [P, NT2 * 8], I16)
            pidx_sb = load(tp, "pidx", p_idx[:], [P, NT3 * 8], I16)

            l1wt_dr = l1_wt[:].rearrange("p (t d) -> p t d", d=P)
            pwt_dr = p_wt[:].rearrange("p (t d) -> p t d", d=P)

            # ---------------- L1: dst-sharded aggregate of x, then W1+relu
            y1_sb = accp.tile([P, BLK, FH], FP8, tag="y1")
            y1_dr = y1_in[:].rearrange("(b p) f -> p b f", p=P)
            for grp in range(NGRP1 if _MAX_PHASE >= 1 else 0):
                TS = SOFF1[grp + 1] - SOFF1[grp]
                msg = mp.tile([P, TS, FIN // 2], F32, tag="msg1")
                nc.gpsimd.dma_gather(
                    msg[:], x_fl[:],
                    l1idx_sb[:, SOFF1[grp] * 8 : SOFF1[grp + 1] * 8],
                    TS * P, TS * P, FIN // 2, single_packet=False,
                )
                wts = wtp.tile([P, TS, P], FP8, tag="wts1")
                nc.sync.dma_start(
                    out=wts[:], in_=l1wt_dr[:, SOFF1[grp] : SOFF1[grp + 1], :]
                )
                toff = 0
                for lb in range(GRP1):
                    b = grp * GRP1 + lb
                    T = T1[b]
                    psum = ps.tile([P, FIN], F32, tag="scat")
                    for t2 in range(T // 2):
                        nc.tensor.matmul(
                            psum[:],
                            wts[:, toff + 2 * t2 : toff + 2 * t2 + 2, :],
                            msg[:, toff + 2 * t2 : toff + 2 * t2 + 2, :].bitcast(FP8)[:, :, 0:FIN],
                            start=(t2 == 0), stop=(T % 2 == 0 and t2 == T // 2 - 1),
                            perf_mode=mybir.MatmulPerfMode.DoubleRow,
                        )
                    if T % 2 == 1:
                        nc.tensor.matmul(
                            psum[:], wts[:, toff + T - 1, :],
                            msg[:, toff + T - 1, :].bitcast(FP8)[:, 0:FIN],
                            start=(T == 1), stop=True,
                        )
                    toff += T
                    # dense: y1 = relu(agg @ W1 + b1)
                    aggb = wk.tile([P, FIN], BF16, tag="aggb1")
                    nc.scalar.copy(aggb[:], psum[:])
                    aggT_ps = pst.tile([P, P], BF16, tag="tr")
                    nc.tensor.transpose(aggT_ps[:], aggb[:], identb_sb[:])
                    aggT = wk.tile([P, P], BF16, tag="aggT1s")
                    nc.vector.tensor_copy(aggT[:], aggT_ps[:])
                    h_ps = psm.tile([P, FH], F32, tag="mm")
                    nc.tensor.matmul(h_ps[:], aggT[:], w1_sb[:], start=True, stop=False)
                    nc.tensor.matmul(
                        h_ps[:], ones_row_sb[:], b1_sb[:], start=False, stop=True
                    )
                    nc.scalar.activation(
                        y1_sb[:, b, :], h_ps[:], mybir.ActivationFunctionType.Relu
                    )
                    nc.scalar.dma_start(out=y1_dr[:, b, :], in_=y1_sb[:, b, :].bitcast(F32))
            if _DEBUG_OUTPUTS and _MAX_PHASE >= 1:
                nc.sync.dma_start(out=dbg["y1"][:], in_=y1_in[:])

            # ---------------- L2: src-sharded partial aggregation + RS
            # one-hot wt tiles are host-precomputed fp8 and streamed from HBM
            rs1_dr = rs1_in[:].rearrange("(g p) f -> p g f", p=P)
            l2wt_dr = l2_wt[:].rearrange("p (t d) -> p t d", d=P)
            for grp in range(NGRP if _MAX_PHASE >= 2 else 0):
                TS = SOFF2[grp + 1] - SOFF2[grp]
                msg = mp2.tile([P, TS, FH // 4], F32, tag="msg2")
                nc.gpsimd.dma_gather(
                    msg[:], y1_in[:],
                    l2idx_sb[:, SOFF2[grp] * 8 : SOFF2[grp + 1] * 8],
                    TS * P, TS * P, FH // 4, single_packet=False,
                )
                wts = wtp.tile([P, TS, P], FP8, tag="wts2")
                nc.sync.dma_start(
                    out=wts[:], in_=l2wt_dr[:, SOFF2[grp] : SOFF2[grp + 1], :]
                )
                ev = evp.tile([P, GRP, FH], BF16, tag="evT")
                toff = 0
                for lb in range(GRP):
                    g = grp * GRP + lb
                    T = T2[g]
                    psum = ps.tile([P, FH], F32, tag="scat")
                    for t2 in range(T // 2):
                        # fp8 DoubleRow: 256 edge slots per matmul
                        nc.tensor.matmul(
                            psum[:],
                            wts[:, toff + 2 * t2 : toff + 2 * t2 + 2, :],
                            msg[:, toff + 2 * t2 : toff + 2 * t2 + 2, :].bitcast(FP8),
                            start=(t2 == 0), stop=(T % 2 == 0 and t2 == T // 2 - 1),
                            perf_mode=mybir.MatmulPerfMode.DoubleRow,
                        )
                    if T % 2 == 1:
                        nc.tensor.matmul(
                            psum[:], wts[:, toff + T - 1, :],
                            msg[:, toff + T - 1, :].bitcast(FP8),
                            start=(T == 1), stop=True,
                        )
                    toff += T
                    if lb == 3:
                        nc.scalar.copy(ev[:, lb, :], psum[:])
                    else:
                        nc.vector.tensor_copy(ev[:, lb, :], psum[:])
                nc.scalar.dma_start(
                    out=rs1_dr[:, grp * GRP : (grp + 1) * GRP, :], in_=ev[:]
                )
            if _MAX_PHASE >= 3:
                RS(rs1_in[:], rs1_out[:])
                if _DEBUG_OUTPUTS:
                    nc.sync.dma_start(out=dbg["agg2"][:], in_=rs1_out[:])

            # ---------------- local dense: y2 = relu(agg@W2+b2); s = softmax
            s_sb = accp.tile([P, BLK, K], F32, tag="s")
            spad_sb = accp.tile([P, BLK, FH], FP8, tag="spad")
            ssq_sb = accp.tile([P, BLK], F32, tag="ssq")
            sscratch = wk.tile([P, K], F32, tag="sscratch")
            s_dr = s_in[:].rearrange("(b p) k -> p b k", p=P)
            if _MAX_PHASE >= 4:
                nc.vector.memset(spad_sb[:], 0.0)
                aggsb = accp.tile([P, BLK, FH], BF16, tag="aggsb")
                rs1o_dr = rs1_out[:].rearrange("(b p) f -> p b f", p=P)
                nc.sync.dma_start(out=aggsb[:, 0:2, :], in_=rs1o_dr[:, 0:2, :])
                nc.sync.dma_start(out=aggsb[:, 2:BLK, :], in_=rs1o_dr[:, 2:BLK, :])
                for b in range(BLK):
                    aggT2 = wk.tile([P, 2, P], BF16, tag="aggT2")
                    for fc in range(2):
                        tps = pst.tile([P, P], BF16, tag="tr")
                        nc.tensor.transpose(
                            tps[:], aggsb[:, b, fc * P : (fc + 1) * P], identb_sb[:]
                        )
                        if fc == 0:
                            nc.vector.tensor_copy(aggT2[:, fc, :], tps[:])
                        else:
                            nc.scalar.copy(aggT2[:, fc, :], tps[:])
                    # h2T[fout, n] = sum_fc W2[fc, fout]^T agg[fc, n]
                    h2t_ps = psm.tile([P, 2, P], F32, tag="mm")
                    for oc in range(2):
                        nc.tensor.matmul(
                            h2t_ps[:, oc, :], b2r_sb[:, oc, :], ones_row_sb[:],
                            start=True, stop=False,
                        )
                        for fc in range(2):
                            nc.tensor.matmul(
                                h2t_ps[:, oc, :],
                                w2_sb[:, fc, oc, :],
                                aggT2[:, fc, :],
                                start=False, stop=(fc == 1),
                            )
                    o2t = wk.tile([P, 2, P], BF16, tag="o2t")
                    nc.vector.tensor_scalar_max(o2t[:], h2t_ps[:], 0.0)
                    sp = psm.tile([P, K], F32, tag="mm")
                    for oc in range(2):
                        nc.tensor.matmul(
                            sp[:], o2t[:, oc, :], wp_sb[:, oc, :],
                            start=(oc == 0), stop=False,
                        )
                    nc.tensor.matmul(
                        sp[:], ones_row_sb[:], bp_sb[:], start=False, stop=True
                    )
                    smax = wk.tile([P, 1], F32, tag="smax")
                    nc.vector.tensor_reduce(
                        smax[:], sp[:], axis=mybir.AxisListType.X,
                        op=mybir.AluOpType.max, negate=True,
                    )
                    sexp = wk.tile([P, K], F32, tag="sexp")
                    ssum = wk.tile([P, 1], F32, tag="ssum")
                    nc.scalar.activation(
                        sexp[:], sp[:], mybir.ActivationFunctionType.Exp,
                        bias=smax[:], accum_out=ssum[:],
                    )
                    nc.vector.reciprocal(ssum[:], ssum[:])
                    nc.vector.tensor_scalar(
                        s_sb[:, b, :], sexp[:], ssum[:], mask_sb[:, b : b + 1],
                        op0=mybir.AluOpType.mult, op1=mybir.AluOpType.mult,
                    )
                    nc.scalar.activation(
                        sscratch[:], s_sb[:, b, :],
                        mybir.ActivationFunctionType.Square,
                        accum_out=ssq_sb[:, b : b + 1],
                    )
                    nc.vector.tensor_copy(spad_sb[:, b, 0:K], s_sb[:, b, :])
                    nc.sync.dma_start(
                        out=s_dr[:, b, :], in_=spad_sb[:, b, :].bitcast(F32)
                    )
                if _DEBUG_OUTPUTS:
                    nc.sync.dma_start(
                        out=dbg["s"][:].rearrange("(b p) k -> p b k", p=P),
                        in_=s_sb[:],
                    )

            # ---- ss/den/trss partials, packed into the RS2 payload
            if _MAX_PHASE >= 4:
                fin_ps = psa.tile([P, K + 8], F32, tag="fin")
                ss_ps = fin_ps[0:K, 0:K]
                smalls = fin_ps[:, K : K + 8]
                for b in range(BLK):
                    nc.tensor.matmul(
                        ss_ps, s_sb[:, b, :], s_sb[:, b, :],
                        start=(b == 0), stop=(b == BLK - 1),
                    )
                den_sb = wk.tile([P, BLK], F32, tag="den")
                nc.vector.tensor_tensor(
                    out=den_sb[:], in0=ssq_sb[:], in1=d_sb[:],
                    op=mybir.AluOpType.mult,
                )
                red2 = wk.tile([P, 1], F32, tag="red2")
                nc.vector.tensor_reduce(
                    red2[:], den_sb[:], axis=mybir.AxisListType.X,
                    op=mybir.AluOpType.add,
                )
                den_ps = smalls[0:1, 1:2]
                nc.tensor.matmul(den_ps, red2[:], ones_sb[:], start=True, stop=True)
                red3 = wk.tile([P, 1], F32, tag="red3")
                nc.vector.tensor_reduce(
                    red3[:], ssq_sb[:], axis=mybir.AxisListType.X,
                    op=mybir.AluOpType.add,
                )
                tr_ps = smalls[0:1, 2:3]
                nc.tensor.matmul(tr_ps, red3[:], ones_sb[:], start=True, stop=True)
                arbuf = wk.tile([K, K + 8], BF16, tag="arbuf")
                nc.vector.memset(arbuf[:], 0.0)
                nc.vector.tensor_copy(arbuf[:, 0:K], ss_ps)
                # hi/lo split of den/trss so the bf16 reduce stays accurate
                hi = wk.tile([1, 2], F32, tag="hi")
                nc.vector.tensor_copy(arbuf[0:1, K + 1 : K + 3], smalls[0:1, 1:3])
                nc.vector.tensor_copy(hi[:], arbuf[0:1, K + 1 : K + 3])
                lo = wk.tile([1, 2], F32, tag="lo")
                nc.vector.tensor_tensor(
                    out=lo[:], in0=smalls[0:1, 1:3], in1=hi[:],
                    op=mybir.AluOpType.subtract,
                )
                nc.vector.tensor_copy(arbuf[0:1, K + 5 : K + 7], lo[:])
                for c_ in range(C):
                    nc.sync.dma_start(
                        out=rs2_in[c_ * CH + SHARD : (c_ + 1) * CH, :], in_=arbuf[:]
                    )

            # ---------------- pool: col-sharded partial adj@s + RS

            for grp in range(NGRP if _MAX_PHASE >= 5 else 0):
                TS = SOFF3[grp + 1] - SOFF3[grp]
                msg = mp2.tile([P, TS, P // 2], F32, tag="msg3")
                nc.gpsimd.dma_gather(
                    msg[:], s_in[:],
                    pidx_sb[:, SOFF3[grp] * 8 : SOFF3[grp + 1] * 8],
                    TS * P, TS * P, P // 2, single_packet=False,
                )
                wts = wtp.tile([P, TS, P], FP8, tag="wts3")
                nc.sync.dma_start(
                    out=wts[:], in_=pwt_dr[:, SOFF3[grp] : SOFF3[grp + 1], :]
                )
                ev = evp.tile([P, GRP, K + 8], BF16, tag="ev3")
                nc.vector.memset(ev[:], 0.0)
                toff = 0
                for lb in range(GRP):
                    g = grp * GRP + lb
                    T = T3[g]
                    psum = ps.tile([P, K], F32, tag="scat")
                    for t2 in range(T // 2):
                        nc.tensor.matmul(
                            psum[:],
                            wts[:, toff + 2 * t2 : toff + 2 * t2 + 2, :],
                            msg[:, toff + 2 * t2 : toff + 2 * t2 + 2, :].bitcast(FP8)[:, :, 0:K],
                            start=(t2 == 0), stop=(T % 2 == 0 and t2 == T // 2 - 1),
                            perf_mode=mybir.MatmulPerfMode.DoubleRow,
                        )
                    if T % 2 == 1:
                        nc.tensor.matmul(
                            psum[:], wts[:, toff + T - 1, :],
                            msg[:, toff + T - 1, :].bitcast(FP8)[:, 0:K],
                            start=(T == 1), stop=True,
                        )
                    toff += T
                    nc.vector.tensor_copy(ev[:, lb, 0:K], psum[:])
                ch = (grp * GRP) // BLK
                row0 = ch * CH + (grp * GRP - ch * BLK) * P
                nc.scalar.dma_start(
                    out=rs2_in[row0 : row0 + GRP * P, :].rearrange(
                        "(g p) k -> p g k", p=P
                    ),
                    in_=ev[:],
                )
            if _MAX_PHASE >= 6:
                RS(rs2_in[:], rs2_out[:])
                if _DEBUG_OUTPUTS:
                    nc.sync.dma_start(out=dbg["adjs"][:], in_=rs2_out[:])

            # ---------------- num partial + tiny AllGather + final scalar
            if _MAX_PHASE >= 7:
                adjsb = accp.tile([P, BLK, K + 8], BF16, tag="adjsb")
                rs2o_dr = rs2_out[0:SHARD, :].rearrange("(b p) k -> p b k", p=P)
                H = BLK // 2
                nc.sync.dma_start(out=adjsb[:, 0:H, :], in_=rs2o_dr[:, 0:H, :])
                nc.sync.dma_start(out=adjsb[:, H:BLK, :], in_=rs2o_dr[:, H:BLK, :])
                accb = wk.tile([K, K + 8], BF16, tag="accb")
                nc.sync.dma_start(out=accb[:], in_=rs2_out[SHARD:CH, :])
                acc = wk.tile([K, K + 8], F32, tag="acc")
                nc.vector.tensor_copy(acc[:], accb[:])
                # re-merge hi+lo scalars
                nc.vector.tensor_tensor(
                    out=acc[0:1, K + 1 : K + 3], in0=acc[0:1, K + 1 : K + 3],
                    in1=acc[0:1, K + 5 : K + 7], op=mybir.AluOpType.add,
                )
                nscratch = accp.tile([P, BLK, K], F32, tag="nscratch")
                nc.vector.tensor_tensor(
                    out=nscratch[:, 0:H, :], in0=s_sb[:, 0:H, :],
                    in1=adjsb[:, 0:H, 0:K], op=mybir.AluOpType.mult,
                )
                nc.vector.tensor_tensor(
                    out=nscratch[:, H:BLK, :], in0=s_sb[:, H:BLK, :],
                    in1=adjsb[:, H:BLK, 0:K], op=mybir.AluOpType.mult,
                )
                red = wk.tile([P, 1], F32, tag="red")
                nc.vector.tensor_reduce(
                    red[:], nscratch[:].rearrange("p b k -> p (b k)"),
                    axis=mybir.AxisListType.X, op=mybir.AluOpType.add,
                )
                num_ps = smalls[0:1, 0:1]
                nc.tensor.matmul(num_ps, red[:], ones_sb[:], start=True, stop=True)
                numbuf = wk.tile([1, 1], F32, tag="numbuf")
                nc.vector.tensor_copy(numbuf[:], num_ps)
                nc.sync.dma_start(out=ar2_in[:], in_=numbuf[:])
                AG(ar2_in[:], ar2_out[:])
                # closed-form ortho from the RS2-reduced scalars (runs under AG)
                ss_tot = acc[:, 0:K]
                if _DEBUG_OUTPUTS:
                    nc.sync.dma_start(out=dbg["ss"][:], in_=ss_tot)
                sq64 = wk.tile([K, K], F32, tag="sq64")
                col64 = wk.tile([K, 1], F32, tag="col64")
                nc.scalar.activation(
                    sq64[:], ss_tot, mybir.ActivationFunctionType.Square,
                    accum_out=col64[:],
                )
                fro_ps = smalls[0:1, 3:4]
                nc.tensor.matmul(fro_ps, col64[:], ones_sb[:K, :], start=True, stop=True)
                rfro = wk.tile([1, 1], F32, tag="rfro")
                nc.scalar.sqrt(rfro[:], fro_ps)
                nc.vector.reciprocal(rfro[:], rfro[:])
                o2 = wk.tile([1, 1], F32, tag="o2s")
                nc.vector.tensor_tensor(
                    out=o2[:], in0=acc[0:1, K + 2 : K + 3], in1=rfro[:],
                    op=mybir.AluOpType.mult,
                )
                nc.vector.tensor_scalar(
                    o2[:], o2[:], -2.0 / float(np.sqrt(K)), 2.0,
                    op0=mybir.AluOpType.mult, op1=mybir.AluOpType.add,
                )
                orth = wk.tile([1, 1], F32, tag="orth_sb")
                nc.scalar.sqrt(orth[:], o2[:])
                rden = wk.tile([1, 1], F32, tag="rden")
                nc.vector.reciprocal(rden[:], acc[0:1, K + 1 : K + 2])
                g2 = wk.tile([1, C], F32, tag="g2")
                nc.sync.dma_start(out=g2[:], in_=ar2_out[:].rearrange("c f -> f c"))
                num_tot = wk.tile([1, 1], F32, tag="num_tot")
                nc.vector.tensor_reduce(
                    num_tot[:], g2[:], axis=mybir.AxisListType.X,
                    op=mybir.AluOpType.add,
                )
                if _DEBUG_OUTPUTS:
                    nd = wk.tile([1, 2], F32, tag="nd")
                    nc.vector.tensor_copy(nd[:, 0:1], num_tot[:])
                    nc.vector.tensor_copy(nd[:, 1:2], acc[0:1, K + 1 : K + 2])
                    nc.sync.dma_start(out=dbg["numden"][:], in_=nd[:])
                mloss = wk.tile([1, 1], F32, tag="mloss")
                nc.vector.tensor_tensor(
                    out=mloss[:], in0=num_tot[:], in1=rden[:],
                    op=mybir.AluOpType.mult,
                )
                res = wk.tile([1, 1], F32, tag="res")
                nc.vector.tensor_tensor(
                    out=res[:], in0=orth[:], in1=mloss[:],
                    op=mybir.AluOpType.subtract,
                )
                nc.sync.dma_start(out=out_t[:], in_=res[:])
            else:
                zz = wk.tile([1, 1], F32, tag="zz")
                nc.vector.memset(zz[:], 0.0)
                nc.sync.dma_start(out=out_t[:], in_=zz[:])

    if not for_sim:
        _split_excess_waits(nc)
    lower_extended_insts(nc)
    return nc


_PROG_CACHE = {}


def prog_key(tabs):
    return (tabs["T1"], tabs["T2"], tabs["T3"])


def _get_program(key, for_sim=False):
    k = (key, for_sim)
    if k not in _PROG_CACHE:
        _PROG_CACHE[k] = build_program(*key, for_sim=for_sim)
    return _PROG_CACHE[k]


def make_in_maps(inputs, tabs):
    x = np.asarray(inputs["x"], np.float32)
    W1, W2, Wp = (np.asarray(inputs[k], np.float32) for k in ("W1", "W2", "Wp"))
    b1, b2, bp = (np.asarray(inputs[k], np.float32) for k in ("b1", "b2", "bp"))
    xpad = np.zeros((NPAD, FIN), np.float32)
    xpad[:N] = x
    identb = np.eye(P, dtype=NPBF16)

    # W2 [256, 256] -> [fin_p, fc, oc, fout_p]: W2[fc*128+p, oc*128+q]
    w2r = np.ascontiguousarray(
        W2.reshape(2, P, 2, P).transpose(1, 0, 2, 3)
    ).astype(NPBF16)
    # Wp [256, 64] -> [fout_p, oc, k]
    wpr = np.ascontiguousarray(Wp.reshape(2, P, K).transpose(1, 0, 2)).astype(NPBF16)
    b2r = np.ascontiguousarray(b2.reshape(1, 2, P)).astype(np.float32)

    common = dict(
        W1=W1.astype(NPBF16),
        W2=w2r,
        Wp=wpr,
        b1=b1.reshape(1, FH).astype(np.float32),
        b2r=b2r,
        bp=bp.reshape(1, K).astype(np.float32),
        identb=identb,
        ones=np.ones((P, 1), np.float32),
        ones_row=np.ones((1, P), np.float32),
        x_full=np.ascontiguousarray(
            np.concatenate(
                [xpad.astype(ml_dtypes.float8_e4m3),
                 np.zeros((NPAD, FIN), ml_dtypes.float8_e4m3)], axis=1,
            )
        ).view(np.float32),
    )
    in_maps = []
    for c in range(C):
        in_maps.append(
            dict(
                common,
                l1_idx=tabs["l1_idx"][c], l1_wt=tabs["l1_wt"][c],
                l2_idx=tabs["l2_idx"][c], l2_wt=tabs["l2_wt"][c],
                p_idx=tabs["p_idx"][c], p_wt=tabs["p_wt"][c],
                d=tabs["d"][c],
                mask=tabs["mask"][c],
            )
        )
    return in_maps


def kernel(x, edge_index, edge_weight, W1, b1, W2, b2, Wp, bp):
    edge_index = np.asarray(edge_index)
    edge_weight = np.asarray(edge_weight, np.float32)
    tabs = preprocess(edge_index, edge_weight)
    nc = _get_program(prog_key(tabs))
    in_maps = make_in_maps(
        dict(x=x, W1=W1, b1=b1, W2=W2, b2=b2, Wp=Wp, bp=bp), tabs
    )
    trace = bool(int(os.environ.get("KERNEL_TRACE", "0")))
    kwargs = {}
    if trace:
        kwargs = dict(trace=True, tmpdir=os.environ.get("KERNEL_TRACE_DIR"))
    res = run_bass_kernel_spmd(nc, in_maps, core_ids=list(range(C)), **kwargs)
    if trace:
        kernel.exec_time_ns = res.exec_time_ns
        kernel.mean_exec_time_ns = res.mean_exec_time_ns
        kernel.bass_results = res
    out = res.results[0]["out"].reshape(())
    if _DEBUG_OUTPUTS:
        kernel.debug = {
            k: [res.results[c].get(f"dbg_{k}") for c in range(C)]
            for k in ("y1", "agg2", "s", "adjs", "numden", "ss")
        }
    return np.float32(out)


if __name__ == "__main__":
    import reference

    inputs = reference.setup_inputs()
    inputs = {k: np.asarray(v) for k, v in inputs.items()}
    got = kernel(**inputs)
    print("kernel out:", got)
